# revision 24
# baseline (speedup 1.0000x reference)
"""Trainium2 Bass kernel for the FFF (fast feedforward / MoE-routing) module.

Math (per token x of dim 1024, PAR=8 trees of 255 nodes):
  logits = x @ W_in.T + b_in                      # [B, 2040]
  dec    = logits > 0
  acts   = silu(logits)
  dmap   = indicator of the 8 visited nodes per tree (root + 7 descents,
           descending by dec at the current node)
  out    = (acts * dmap) @ W_out.T                # [B, 1024]

Strategy (8 NeuronCores, data-parallel over the 8192 tokens, 1024 each):
  - GEMM1 in fp16 with region-dependent precision.  A decision flip at
    depth d corrupts 7-d downstream activations, so only the shallow
    nodes (0..15, levels 0..3) get the expensive treatment:
      cols   0..128 (nodes  0..15): x_hi*w + x_lo*w + x_hi*w_lo  (~fp32)
      cols 128..2040 (everything else): single x_hi*w pass
    The w_lo correction weights are pre-scaled by 2^10 (and x_hi by
    2^-10 on-device) so they stay in fp16 normal range.  fp32 bias is
    added on the vector engine.  Measured on the reference data this
    gives ~1.35e-2 overall rel err (gate is 2e-2).
  - dmap is built level-by-level with strided vector ops in a node-major
    column layout (col = 8*node + tree): child1 = V_d * dec_d (stride-2
    upsample), child0 = V_d - child1.
  - masked acts cast to fp16, transposed 128x128 on the PE, GEMM2 in fp16
    (exact products, fp32 PSUM accumulation).
  - startup: the 4.7MB of GEMM1 weights stream in as four 512-column
    slabs (one dma_start each; the DGE fair-shares ~300GB/s across
    in-flight dispatches, so fewer+ordered dispatches = earliest slab0).
    Tiles 0 and 1 are processed SLAB-MAJOR so each arriving slab feeds
    2 tiles of PE work; the fp32 bias arrives as fp16 (hi, 2^10*lo) rows
    and is broadcast across partitions on the PE (K=2 matmul against a
    (1, 2^-10) column pair -> exact fp32 in PSUM) during the initial
    weight wait instead of a 1MB broadcast DMA competing for early
    bandwidth.
"""

import numpy as np

DIM = 1024
PAR = 8
DEPTH = 7
N_NODES = 255
WIDTH = PAR * N_NODES          # 2040
NODES_PAD = 2048               # pad masked-acts/W_out^T to 16*128
N_CORES = 8
TOK_PER_CORE = 1024
TT = 128                       # tokens per tile
NTILES = TOK_PER_CORE // TT    # 8
K_CH = DIM // 128              # 8 contraction chunks for GEMM1
C_CH = NODES_PAD // 128        # 16 contraction chunks for GEMM2
DEC_COLS = 8 * 127             # 1016: decision nodes are levels 0..6
SH_COLS = 128                  # hi/lo-corrected region: nodes 0..15
LO_SCALE = 1024.0              # 2^10 keeps w_lo out of fp16 subnormals
N_SLAB = 4                     # w1 column slabs of 512 (last is 504+pad)

_PROGRAM = None


def _build_program():
    import concourse.bacc as bacc
    import concourse.tile as tile
    from concourse import mybir
    from concourse.masks import make_identity
    import concourse.bass as bass

    f32 = mybir.dt.float32
    f16 = mybir.dt.float16
    Alu = mybir.AluOpType
    Act = mybir.ActivationFunctionType

    nc = bacc.Bacc("TRN2", target_bir_lowering=False, debug=False,
                   num_devices=N_CORES)

    xt = nc.dram_tensor("xt", [128, NTILES, 2, K_CH, TT], f16,
                        kind="ExternalInput")
    w1 = nc.dram_tensor("w1", [128, N_SLAB, K_CH, 512], f16,
                        kind="ExternalInput")
    w1l = nc.dram_tensor("w1l", [128, K_CH, SH_COLS], f16,
                         kind="ExternalInput")
    b1hl = nc.dram_tensor("b1hl", [2, WIDTH], f16, kind="ExternalInput")
    ones2 = nc.dram_tensor("ones2", [2, 128], f16, kind="ExternalInput")
    w2 = nc.dram_tensor("w2", [128, C_CH, DIM], f16, kind="ExternalInput")
    y = nc.dram_tensor("y", [TOK_PER_CORE, DIM], f32, kind="ExternalOutput")

    SLAB_LIM = [(0, 512), (512, 1024), (1024, 1536), (1536, WIDTH)]

    with tile.TileContext(nc) as tc:
        with (
            tc.tile_pool(name="wts", bufs=1) as wts,
            tc.tile_pool(name="xts", bufs=4) as xts,
            tc.tile_pool(name="xh2s", bufs=2) as xh2s,
            tc.tile_pool(name="logits", bufs=2) as logits_pool,
            tc.tile_pool(name="mask", bufs=2) as mask_pool,
            tc.tile_pool(name="acts", bufs=2) as acts_pool,
            tc.tile_pool(name="mks", bufs=3) as mks_pool,
            tc.tile_pool(name="out", bufs=2) as out_pool,
            tc.tile_pool(name="pl", bufs=3, space="PSUM") as pl_pool,
            tc.tile_pool(name="pt", bufs=3, space="PSUM") as pt_pool,
            tc.tile_pool(name="py", bufs=2, space="PSUM") as py_pool,
        ):
            # ---- resident weights ----
            w1_sb = wts.tile([128, N_SLAB, K_CH, 512], f16)
            w1l_sb = wts.tile([128, K_CH, SH_COLS], f16)
            w2_sb = wts.tile([128, C_CH, DIM], f16)
            b1_sb = wts.tile([128, WIDTH], f32)
            b1_row = wts.tile([2, WIDTH], f16)
            ones = wts.tile([2, 128], f16)
            ident = wts.tile([128, 128], f16)

            xt_tiles = {}

            def prefetch_xt(j):
                xhl = xts.tile([128, 2, K_CH, TT], f16, tag="x")
                nc.sync.dma_start(out=xhl, in_=xt[:, j, :, :, :])
                xt_tiles[j] = xhl

            # Startup DMAs on the Sync engine in PE consumption order.
            # The DGE fair-shares bandwidth over in-flight dispatches and
            # completes them in dispatch order, so this order == arrival
            # order.
            nc.gpsimd.dma_start(out=ones, in_=ones2[:, :])
            nc.gpsimd.dma_start(out=b1_row, in_=b1hl[:, :])
            xhl0 = xts.tile([128, 2, K_CH, TT], f16, tag="x")
            xhl1 = xts.tile([128, 2, K_CH, TT], f16, tag="x")
            nc.sync.dma_start(out=xhl0[:, 0], in_=xt[:, 0, 0, :, :])
            nc.sync.dma_start(out=w1_sb[:, 0, 0:4], in_=w1[:, 0, 0:4])
            nc.sync.dma_start(out=xhl0[:, 1], in_=xt[:, 0, 1, :, :])
            nc.sync.dma_start(out=w1_sb[:, 0, 4:8], in_=w1[:, 0, 4:8])
            nc.sync.dma_start(out=w1l_sb, in_=w1l[:, :, :])
            nc.sync.dma_start(out=xhl1[:, 0], in_=xt[:, 1, 0, :, :])
            nc.sync.dma_start(out=xhl1[:, 1], in_=xt[:, 1, 1, :, :])
            xt_tiles[0] = xhl0
            xt_tiles[1] = xhl1
            for s in range(1, N_SLAB):
                nc.sync.dma_start(out=w1_sb[:, s], in_=w1[:, s])
            prefetch_xt(2)
            nc.sync.dma_start(out=w2_sb[:, 0:4, :], in_=w2[:, 0:4, :])
            prefetch_xt(3)
            nc.sync.dma_start(out=w2_sb[:, 4:8, :], in_=w2[:, 4:8, :])
            nc.sync.dma_start(out=w2_sb[:, 8:12, :], in_=w2[:, 8:12, :])
            nc.sync.dma_start(out=w2_sb[:, 12:16, :], in_=w2[:, 12:16, :])
            make_identity(nc, ident)

            # bias broadcast across partitions on the PE while weights
            # stream in: b1 comes as fp16 (hi, 2^10*lo) rows, contracted
            # (K=2) against (1, 2^-10) -> exact fp32 bias in PSUM.
            for c0, c1 in SLAB_LIM:
                pb = pl_pool.tile([TT, 512], f32, tag="pl")
                nc.tensor.matmul(pb[:, 0:c1 - c0], lhsT=ones,
                                 rhs=b1_row[:, c0:c1], start=True, stop=True)
                nc.vector.tensor_copy(b1_sb[:, c0:c1], pb[:, 0:c1 - c0])

            state = {}

            def epilogue_vec(j, lg, d1, vv, ac):
                # tree mask: V_0 = 1 at root cols; then per level
                # child1 = V_d * dec_d, child0 = V_d - child1
                nc.vector.memset(vv[:, 0:8], 1.0)
                for d in range(DEPTH):
                    ld = 8 * (1 << d)
                    c0 = 8 * ((1 << d) - 1)
                    c1 = 8 * ((1 << (d + 1)) - 1)
                    vpar = vv[:, c0:c0 + ld].rearrange("p (i t) -> p i t", t=8)
                    dpar = d1[:, c0:c0 + ld].rearrange("p (i t) -> p i t", t=8)
                    kids = vv[:, c1:c1 + 2 * ld].rearrange(
                        "p (i two t) -> p i two t", two=2, t=8)
                    nc.vector.tensor_tensor(kids[:, :, 1, :], vpar, dpar,
                                            Alu.mult)
                    nc.vector.tensor_tensor(kids[:, :, 0, :], vpar,
                                            kids[:, :, 1, :], Alu.subtract)

            def finish_mask(j, ac, vv):
                mk = mks_pool.tile([TT, NODES_PAD], f16, tag="mk")
                nc.vector.memset(mk[:, WIDTH:NODES_PAD], 0.0)
                nc.vector.tensor_tensor(mk[:, 0:1024], ac[:, 0:1024],
                                        vv[:, 0:1024], Alu.mult)
                nc.vector.tensor_tensor(mk[:, 1024:WIDTH], ac[:, 1024:WIDTH],
                                        vv[:, 1024:WIDTH], Alu.mult)
                state[j] = mk

            def gemm1_slab(s, xh, xl, xh2, lg, d1, ac):
                """Matmuls + per-slab epilogue for slab s of one tile."""
                c0, c1 = SLAB_LIM[s]
                w = c1 - c0
                p = pl_pool.tile([TT, 512], f32, tag="pl")
                if s == 0:
                    for k in range(K_CH):
                        nc.tensor.matmul(p, lhsT=xh[:, k, :],
                                         rhs=w1_sb[:, 0, k, :],
                                         start=(k == 0), stop=False)
                    for k in range(K_CH):
                        nc.tensor.matmul(p[:, 0:SH_COLS], lhsT=xl[:, k, :],
                                         rhs=w1_sb[:, 0, k, 0:SH_COLS],
                                         start=False, stop=False)
                    for k in range(K_CH):
                        nc.tensor.matmul(p[:, 0:SH_COLS], lhsT=xh2[:, k, :],
                                         rhs=w1l_sb[:, k, :],
                                         start=False, stop=(k == K_CH - 1))
                else:
                    for k in range(K_CH):
                        nc.tensor.matmul(p[:, 0:w], lhsT=xh[:, k, :],
                                         rhs=w1_sb[:, s, k, 0:w],
                                         start=(k == 0),
                                         stop=(k == K_CH - 1))
                nc.vector.tensor_tensor(lg[:, c0:c1], p[:, 0:w],
                                        b1_sb[:, c0:c1], Alu.add)
                if s == 0:
                    nc.vector.tensor_scalar(d1[:, 0:512], lg[:, 0:512], 0.0,
                                            None, Alu.is_gt)
                elif s == 1:
                    nc.vector.tensor_scalar(d1[:, 512:DEC_COLS],
                                            lg[:, 512:DEC_COLS], 0.0,
                                            None, Alu.is_gt)
                nc.scalar.activation(ac[:, c0:c1], lg[:, c0:c1], Act.Silu)

            def tile_bufs(j):
                xhl = xt_tiles.pop(j)
                xh, xl = xhl[:, 0], xhl[:, 1]
                xh2 = xh2s.tile([128, K_CH, TT], f16, tag="xh2")
                nc.vector.tensor_scalar(xh2, xh, 1.0 / LO_SCALE, None,
                                        Alu.mult)
                lg = logits_pool.tile([TT, WIDTH], f32, tag="lg")
                d1 = mask_pool.tile([TT, DEC_COLS], f16, tag="d1")
                vv = mask_pool.tile([TT, WIDTH], f16, tag="vv")
                ac = acts_pool.tile([TT, WIDTH], f16, tag="ac")
                return xh, xl, xh2, lg, d1, vv, ac

            def stage_a(j):
                if j + 1 < NTILES and j + 1 not in xt_tiles:
                    prefetch_xt(j + 1)
                xh, xl, xh2, lg, d1, vv, ac = tile_bufs(j)
                for s in range(N_SLAB):
                    gemm1_slab(s, xh, xl, xh2, lg, d1, ac)
                epilogue_vec(j, lg, d1, vv, ac)
                finish_mask(j, ac, vv)

            def stage_ab01():
                # tiles 0 and 1 slab-major: each arriving w1 slab feeds
                # 2 tiles of PE work, halving the DMA-bound startup.
                b0 = tile_bufs(0)
                b1_ = tile_bufs(1)
                for s in range(N_SLAB):
                    gemm1_slab(s, b0[0], b0[1], b0[2], b0[3], b0[4], b0[6])
                    gemm1_slab(s, b1_[0], b1_[1], b1_[2], b1_[3], b1_[4],
                               b1_[6])
                    if s == 1:
                        epilogue_vec(0, b0[3], b0[4], b0[5], b0[6])
                        epilogue_vec(1, b1_[3], b1_[4], b1_[5], b1_[6])
                finish_mask(0, b0[6], b0[5])
                finish_mask(1, b1_[6], b1_[5])

            def stage_b(j, last=False):
                mk = state.pop(j)
                at = acts_pool.tile([128, C_CH, TT], f16, tag="at")
                c = 0
                for gsz in (1, 3, 4, 4, 4):
                    pt = pt_pool.tile([128, 512], f16)
                    for i in range(gsz):
                        nc.tensor.transpose(
                            pt[:, i * 128:(i + 1) * 128],
                            mk[:, (c + i) * 128:(c + i + 1) * 128], ident)
                    nc.scalar.copy(
                        at[:, c:c + gsz, :],
                        pt[:, :gsz * 128].rearrange("p (c t) -> p c t", t=TT))
                    c += gsz
                ys = out_pool.tile([TT, DIM], f32, tag="ys")
                py0 = py_pool.tile([TT, 512], f32, tag="py")
                py1 = py_pool.tile([TT, 512], f32, tag="py")
                if last:
                    # serialize the halves so the first store drains while
                    # the second half is still on the PE (shorter tail)
                    for h, py in ((0, py0), (1, py1)):
                        hs = slice(h * 512, (h + 1) * 512)
                        for c in range(C_CH):
                            nc.tensor.matmul(py, lhsT=at[:, c, :],
                                             rhs=w2_sb[:, c, hs],
                                             start=(c == 0),
                                             stop=(c == C_CH - 1))
                        nc.vector.tensor_copy(ys[:, hs], py)
                        nc.sync.dma_start(out=y[j * TT:(j + 1) * TT, hs],
                                          in_=ys[:, hs])
                    return
                # c-outer so w2 chunks are consumed in arrival order
                for c in range(C_CH):
                    nc.tensor.matmul(py0, lhsT=at[:, c, :],
                                     rhs=w2_sb[:, c, 0:512],
                                     start=(c == 0), stop=(c == C_CH - 1))
                    nc.tensor.matmul(py1, lhsT=at[:, c, :],
                                     rhs=w2_sb[:, c, 512:1024],
                                     start=(c == 0), stop=(c == C_CH - 1))
                for h, py in ((0, py0), (1, py1)):
                    hs = slice(h * 512, (h + 1) * 512)
                    nc.vector.tensor_copy(ys[:, hs], py)
                    nc.sync.dma_start(out=y[j * TT:(j + 1) * TT, hs],
                                      in_=ys[:, hs])

            # pipeline: AB(0,1), A(2), B(0), A(3), B(1), ... A(7), B(5),
            # B(6), B(7)
            stage_ab01()
            for j in range(2, NTILES):
                stage_a(j)
                stage_b(j - 2)
            stage_b(NTILES - 2)
            stage_b(NTILES - 1, last=True)

    nc.finalize()
    return nc


def _get_program():
    global _PROGRAM
    if _PROGRAM is None:
        _PROGRAM = _build_program()
    return _PROGRAM


def _split_hi_lo_f16(a):
    hi = a.astype(np.float16)
    lo = (a - hi.astype(np.float32)).astype(np.float16)
    return hi, lo


def kernel(oldx, W_in, b_in, W_out):
    from concourse.bass_utils import run_bass_kernel_spmd

    oldx = np.asarray(oldx)
    W_in = np.asarray(W_in, dtype=np.float32)
    b_in = np.asarray(b_in, dtype=np.float32)
    W_out = np.asarray(W_out, dtype=np.float32)
    x = oldx.reshape(-1, DIM).astype(np.float32)          # [8192, 1024]

    # node-major column permutation: our col 8n+t  <-  ref col 255t+n
    i = np.arange(WIDTH)
    perm = 255 * (i % PAR) + (i // PAR)

    w1t = W_in[perm, :].T.astype(np.float32)              # [1024, 2040]
    w1t_hi = w1t.astype(np.float16)
    w1t_lo = ((w1t - w1t_hi.astype(np.float32)) * LO_SCALE).astype(np.float16)
    # [dim, width] -> [128, N_SLAB, K_CH, 512] with dim = k*128 + p,
    # width col = 512*slab + c (last slab zero-padded to 512)
    w1p = np.zeros((1024, N_SLAB * 512), np.float16)
    w1p[:, :WIDTH] = w1t_hi
    w1 = np.ascontiguousarray(
        w1p.reshape(K_CH, 128, N_SLAB, 512).transpose(1, 2, 0, 3))
    w1l = np.ascontiguousarray(
        w1t_lo.reshape(K_CH, 128, WIDTH).transpose(1, 0, 2)[:, :, :SH_COLS])
    b1p = b_in[perm].astype(np.float32)
    b1h = b1p.astype(np.float16)
    b1hl = np.ascontiguousarray(np.stack(
        [b1h, ((b1p - b1h.astype(np.float32)) * LO_SCALE).astype(np.float16)]))

    w2t = np.zeros((NODES_PAD, DIM), np.float32)
    w2t[:WIDTH] = W_out.T[perm, :]
    w2 = np.ascontiguousarray(
        w2t.astype(np.float16).reshape(C_CH, 128, DIM).transpose(1, 0, 2))
    ones2 = np.ascontiguousarray(np.stack(
        [np.full(128, 1.0, np.float16),
         np.full(128, 1.0 / LO_SCALE, np.float16)]))

    in_maps = []
    for c in range(N_CORES):
        xc = x[c * TOK_PER_CORE:(c + 1) * TOK_PER_CORE]   # [1024, 1024]
        xt_hi, xt_lo = _split_hi_lo_f16(xc.T)             # [dim, tok]
        # [dim, tok] -> [128, NTILES, K_CH, TT]; dim = k*128+p, tok = j*128+t
        xt_hi = xt_hi.reshape(K_CH, 128, NTILES, TT).transpose(1, 2, 0, 3)
        xt_lo = xt_lo.reshape(K_CH, 128, NTILES, TT).transpose(1, 2, 0, 3)
        xt = np.ascontiguousarray(np.stack([xt_hi, xt_lo], axis=2))
        in_maps.append({
            "xt": xt, "w1": w1, "w1l": w1l,
            "b1hl": b1hl, "w2": w2, "ones2": ones2,
        })

    nc = _get_program()
    res = run_bass_kernel_spmd(nc, in_maps, core_ids=list(range(N_CORES)))
    out = np.concatenate([res.results[c]["y"] for c in range(N_CORES)],
                         axis=0)
    return out.reshape(oldx.shape).astype(np.float32)


# revision 26
# speedup vs baseline: 1.0018x; 1.0018x over previous
"""Trainium2 Bass kernel for the FFF (fast feedforward / MoE-routing) module.

Math (per token x of dim 1024, PAR=8 trees of 255 nodes):
  logits = x @ W_in.T + b_in                      # [B, 2040]
  dec    = logits > 0
  acts   = silu(logits)
  dmap   = indicator of the 8 visited nodes per tree (root + 7 descents,
           descending by dec at the current node)
  out    = (acts * dmap) @ W_out.T                # [B, 1024]

Strategy (8 NeuronCores, data-parallel over the 8192 tokens, 1024 each):
  - GEMM1 in fp16 with region-dependent precision.  A decision flip at
    depth d corrupts 7-d downstream activations, so only the shallow
    nodes (0..15, levels 0..3) get the expensive treatment:
      cols   0..128 (nodes  0..15): x_hi*w + x_lo*w + x_hi*w_lo  (~fp32)
      cols 128..2040 (everything else): single x_hi*w pass
    The w_lo correction weights are pre-scaled by 2^10 (and x_hi by
    2^-10 on-device) so they stay in fp16 normal range.  fp32 bias is
    added on the vector engine.  Measured on the reference data this
    gives ~1.35e-2 overall rel err (gate is 2e-2).
  - dmap is built level-by-level with strided vector ops in a node-major
    column layout (col = 8*node + tree): child1 = V_d * dec_d (stride-2
    upsample), child0 = V_d - child1.
  - masked acts cast to fp16, transposed 128x128 on the PE, GEMM2 in fp16
    (exact products, fp32 PSUM accumulation).
  - startup: the 4.7MB of GEMM1 weights stream in as four 512-column
    slabs (one dma_start each; the DGE fair-shares ~300GB/s across
    in-flight dispatches, so fewer+ordered dispatches = earliest slab0).
    Tiles 0 and 1 are processed SLAB-MAJOR so each arriving slab feeds
    2 tiles of PE work; the fp32 bias arrives as fp16 (hi, 2^10*lo) rows
    and is broadcast across partitions on the PE (K=2 matmul against a
    (1, 2^-10) column pair -> exact fp32 in PSUM) during the initial
    weight wait instead of a 1MB broadcast DMA competing for early
    bandwidth.
"""

import numpy as np

DIM = 1024
PAR = 8
DEPTH = 7
N_NODES = 255
WIDTH = PAR * N_NODES          # 2040
NODES_PAD = 2048               # pad masked-acts/W_out^T to 16*128
N_CORES = 8
TOK_PER_CORE = 1024
TT = 128                       # tokens per tile
NTILES = TOK_PER_CORE // TT    # 8
K_CH = DIM // 128              # 8 contraction chunks for GEMM1
C_CH = NODES_PAD // 128        # 16 contraction chunks for GEMM2
DEC_COLS = 8 * 127             # 1016: decision nodes are levels 0..6
SH_COLS = 128                  # hi/lo-corrected region: nodes 0..15
LO_SCALE = 1024.0              # 2^10 keeps w_lo out of fp16 subnormals
N_SLAB = 4                     # w1 column slabs of 512 (last is 504+pad)

_PROGRAM = None


def _build_program():
    import concourse.bacc as bacc
    import concourse.tile as tile
    from concourse import mybir
    from concourse.masks import make_identity
    import concourse.bass as bass

    f32 = mybir.dt.float32
    f16 = mybir.dt.float16
    Alu = mybir.AluOpType
    Act = mybir.ActivationFunctionType

    nc = bacc.Bacc("TRN2", target_bir_lowering=False, debug=False,
                   num_devices=N_CORES)

    xt = nc.dram_tensor("xt", [128, NTILES, 2, K_CH, TT], f16,
                        kind="ExternalInput")
    w1 = nc.dram_tensor("w1", [128, N_SLAB, K_CH, 512], f16,
                        kind="ExternalInput")
    w1l = nc.dram_tensor("w1l", [128, K_CH, SH_COLS], f16,
                         kind="ExternalInput")
    b1hl = nc.dram_tensor("b1hl", [2, WIDTH], f16, kind="ExternalInput")
    ones2 = nc.dram_tensor("ones2", [2, 128], f16, kind="ExternalInput")
    w2 = nc.dram_tensor("w2", [128, C_CH, DIM], f16, kind="ExternalInput")
    y = nc.dram_tensor("y", [TOK_PER_CORE, DIM], f32, kind="ExternalOutput")

    SLAB_LIM = [(0, 512), (512, 1024), (1024, 1536), (1536, WIDTH)]

    with tile.TileContext(nc) as tc:
        with (
            tc.tile_pool(name="wts", bufs=1) as wts,
            tc.tile_pool(name="xts", bufs=4) as xts,
            tc.tile_pool(name="xh2s", bufs=2) as xh2s,
            tc.tile_pool(name="logits", bufs=2) as logits_pool,
            tc.tile_pool(name="mask", bufs=2) as mask_pool,
            tc.tile_pool(name="acts", bufs=2) as acts_pool,
            tc.tile_pool(name="mks", bufs=3) as mks_pool,
            tc.tile_pool(name="out", bufs=2) as out_pool,
            tc.tile_pool(name="pl", bufs=3, space="PSUM") as pl_pool,
            tc.tile_pool(name="pt", bufs=3, space="PSUM") as pt_pool,
            tc.tile_pool(name="py", bufs=2, space="PSUM") as py_pool,
        ):
            # ---- resident weights ----
            w1_sb = wts.tile([128, N_SLAB, K_CH, 512], f16)
            w1l_sb = wts.tile([128, K_CH, SH_COLS], f16)
            w2_sb = wts.tile([128, C_CH, DIM], f16)
            b1_sb = wts.tile([128, WIDTH], f32)
            b1_row = wts.tile([2, WIDTH], f16)
            ones = wts.tile([2, 128], f16)
            ident = wts.tile([128, 128], f16)

            xt_tiles = {}

            def prefetch_xt(j):
                xhl = xts.tile([128, 2, K_CH, TT], f16, tag="x")
                nc.sync.dma_start(out=xhl, in_=xt[:, j, :, :, :])
                xt_tiles[j] = xhl

            # Startup DMAs on the Sync engine in PE consumption order.
            # The DGE fair-shares bandwidth over in-flight dispatches and
            # completes them in dispatch order, so this order == arrival
            # order.
            nc.gpsimd.dma_start(out=ones, in_=ones2[:, :])
            nc.gpsimd.dma_start(out=b1_row, in_=b1hl[:, :])
            xhl0 = xts.tile([128, 2, K_CH, TT], f16, tag="x")
            xhl1 = xts.tile([128, 2, K_CH, TT], f16, tag="x")
            nc.sync.dma_start(out=xhl0[:, 0], in_=xt[:, 0, 0, :, :])
            nc.sync.dma_start(out=w1_sb[:, 0, 0:4], in_=w1[:, 0, 0:4])
            nc.sync.dma_start(out=xhl0[:, 1], in_=xt[:, 0, 1, :, :])
            nc.sync.dma_start(out=w1_sb[:, 0, 4:8], in_=w1[:, 0, 4:8])
            nc.sync.dma_start(out=w1l_sb, in_=w1l[:, :, :])
            nc.sync.dma_start(out=xhl1[:, 0], in_=xt[:, 1, 0, :, :])
            nc.sync.dma_start(out=xhl1[:, 1], in_=xt[:, 1, 1, :, :])
            xt_tiles[0] = xhl0
            xt_tiles[1] = xhl1
            for s in range(1, N_SLAB):
                nc.sync.dma_start(out=w1_sb[:, s], in_=w1[:, s])
            prefetch_xt(2)
            nc.sync.dma_start(out=w2_sb[:, 0:4, :], in_=w2[:, 0:4, :])
            prefetch_xt(3)
            nc.sync.dma_start(out=w2_sb[:, 4:8, :], in_=w2[:, 4:8, :])
            nc.sync.dma_start(out=w2_sb[:, 8:12, :], in_=w2[:, 8:12, :])
            nc.sync.dma_start(out=w2_sb[:, 12:16, :], in_=w2[:, 12:16, :])
            make_identity(nc, ident)

            # bias broadcast across partitions on the PE: fp16 (hi,
            # 2^10*lo) rows contracted (K=2) against (1, 2^-10) -> exact
            # fp32 in PSUM.  Emitted per-region inside stage_ab01, right
            # after each slab's first tile, so the PE's first instruction
            # is GEMM1 itself (gated only on x + slab0) and each bias mm
            # slots into a DMA gap.
            def bias_region(s):
                c0, c1 = SLAB_LIM[s]
                pb = pl_pool.tile([TT, 512], f32, tag="pl")
                nc.tensor.matmul(pb[:, 0:c1 - c0], lhsT=ones,
                                 rhs=b1_row[:, c0:c1], start=True, stop=True)
                nc.vector.tensor_copy(b1_sb[:, c0:c1], pb[:, 0:c1 - c0])

            state = {}

            def epilogue_vec(j, lg, d1, vv, ac):
                # tree mask: V_0 = 1 at root cols; then per level
                # child1 = V_d * dec_d, child0 = V_d - child1
                nc.vector.memset(vv[:, 0:8], 1.0)
                for d in range(DEPTH):
                    ld = 8 * (1 << d)
                    c0 = 8 * ((1 << d) - 1)
                    c1 = 8 * ((1 << (d + 1)) - 1)
                    vpar = vv[:, c0:c0 + ld].rearrange("p (i t) -> p i t", t=8)
                    dpar = d1[:, c0:c0 + ld].rearrange("p (i t) -> p i t", t=8)
                    kids = vv[:, c1:c1 + 2 * ld].rearrange(
                        "p (i two t) -> p i two t", two=2, t=8)
                    nc.vector.tensor_tensor(kids[:, :, 1, :], vpar, dpar,
                                            Alu.mult)
                    nc.vector.tensor_tensor(kids[:, :, 0, :], vpar,
                                            kids[:, :, 1, :], Alu.subtract)

            def finish_mask(j, ac, vv):
                mk = mks_pool.tile([TT, NODES_PAD], f16, tag="mk")
                nc.vector.memset(mk[:, WIDTH:NODES_PAD], 0.0)
                nc.vector.tensor_tensor(mk[:, 0:1024], ac[:, 0:1024],
                                        vv[:, 0:1024], Alu.mult)
                nc.vector.tensor_tensor(mk[:, 1024:WIDTH], ac[:, 1024:WIDTH],
                                        vv[:, 1024:WIDTH], Alu.mult)
                state[j] = mk

            def gemm1_slab_mm(s, xh, xl, xh2):
                c0, c1 = SLAB_LIM[s]
                w = c1 - c0
                p = pl_pool.tile([TT, 512], f32, tag="pl")
                if s == 0:
                    for k in range(K_CH):
                        nc.tensor.matmul(p, lhsT=xh[:, k, :],
                                         rhs=w1_sb[:, 0, k, :],
                                         start=(k == 0), stop=False)
                    for k in range(K_CH):
                        nc.tensor.matmul(p[:, 0:SH_COLS], lhsT=xl[:, k, :],
                                         rhs=w1_sb[:, 0, k, 0:SH_COLS],
                                         start=False, stop=False)
                    for k in range(K_CH):
                        nc.tensor.matmul(p[:, 0:SH_COLS], lhsT=xh2[:, k, :],
                                         rhs=w1l_sb[:, k, :],
                                         start=False, stop=(k == K_CH - 1))
                else:
                    for k in range(K_CH):
                        nc.tensor.matmul(p[:, 0:w], lhsT=xh[:, k, :],
                                         rhs=w1_sb[:, s, k, 0:w],
                                         start=(k == 0),
                                         stop=(k == K_CH - 1))
                return p

            def gemm1_slab_post(s, p, lg, d1, ac):
                c0, c1 = SLAB_LIM[s]
                w = c1 - c0
                nc.vector.tensor_tensor(lg[:, c0:c1], p[:, 0:w],
                                        b1_sb[:, c0:c1], Alu.add)
                if s == 0:
                    nc.vector.tensor_scalar(d1[:, 0:512], lg[:, 0:512], 0.0,
                                            None, Alu.is_gt)
                elif s == 1:
                    nc.vector.tensor_scalar(d1[:, 512:DEC_COLS],
                                            lg[:, 512:DEC_COLS], 0.0,
                                            None, Alu.is_gt)
                nc.scalar.activation(ac[:, c0:c1], lg[:, c0:c1], Act.Silu)

            def gemm1_slab(s, xh, xl, xh2, lg, d1, ac):
                p = gemm1_slab_mm(s, xh, xl, xh2)
                gemm1_slab_post(s, p, lg, d1, ac)

            def tile_bufs(j):
                xhl = xt_tiles.pop(j)
                xh, xl = xhl[:, 0], xhl[:, 1]
                xh2 = xh2s.tile([128, K_CH, TT], f16, tag="xh2")
                nc.vector.tensor_scalar(xh2, xh, 1.0 / LO_SCALE, None,
                                        Alu.mult)
                lg = logits_pool.tile([TT, WIDTH], f32, tag="lg")
                d1 = mask_pool.tile([TT, DEC_COLS], f16, tag="d1")
                vv = mask_pool.tile([TT, WIDTH], f16, tag="vv")
                ac = acts_pool.tile([TT, WIDTH], f16, tag="ac")
                return xh, xl, xh2, lg, d1, vv, ac

            def stage_a(j):
                if j + 1 < NTILES and j + 1 not in xt_tiles:
                    prefetch_xt(j + 1)
                xh, xl, xh2, lg, d1, vv, ac = tile_bufs(j)
                for s in range(N_SLAB):
                    gemm1_slab(s, xh, xl, xh2, lg, d1, ac)
                epilogue_vec(j, lg, d1, vv, ac)
                finish_mask(j, ac, vv)

            def stage_ab01():
                # tiles 0 and 1 slab-major: each arriving w1 slab feeds
                # 2 tiles of PE work, halving the DMA-bound startup.
                b0 = tile_bufs(0)
                b1_ = tile_bufs(1)
                for s in range(N_SLAB):
                    p0 = gemm1_slab_mm(s, b0[0], b0[1], b0[2])
                    bias_region(s)
                    gemm1_slab_post(s, p0, b0[3], b0[4], b0[6])
                    gemm1_slab(s, b1_[0], b1_[1], b1_[2], b1_[3], b1_[4],
                               b1_[6])
                    if s == 1:
                        epilogue_vec(0, b0[3], b0[4], b0[5], b0[6])
                        epilogue_vec(1, b1_[3], b1_[4], b1_[5], b1_[6])
                finish_mask(0, b0[6], b0[5])
                finish_mask(1, b1_[6], b1_[5])

            def stage_b(j, last=False):
                mk = state.pop(j)
                at = acts_pool.tile([128, C_CH, TT], f16, tag="at")
                c = 0
                for gsz in (1, 3, 4, 4, 4):
                    pt = pt_pool.tile([128, 512], f16)
                    for i in range(gsz):
                        nc.tensor.transpose(
                            pt[:, i * 128:(i + 1) * 128],
                            mk[:, (c + i) * 128:(c + i + 1) * 128], ident)
                    nc.scalar.copy(
                        at[:, c:c + gsz, :],
                        pt[:, :gsz * 128].rearrange("p (c t) -> p c t", t=TT))
                    c += gsz
                ys = out_pool.tile([TT, DIM], f32, tag="ys")
                py0 = py_pool.tile([TT, 512], f32, tag="py")
                py1 = py_pool.tile([TT, 512], f32, tag="py")
                if last:
                    # serialize the halves so the first store drains while
                    # the second half is still on the PE (shorter tail)
                    for h, py in ((0, py0), (1, py1)):
                        hs = slice(h * 512, (h + 1) * 512)
                        for c in range(C_CH):
                            nc.tensor.matmul(py, lhsT=at[:, c, :],
                                             rhs=w2_sb[:, c, hs],
                                             start=(c == 0),
                                             stop=(c == C_CH - 1))
                        nc.vector.tensor_copy(ys[:, hs], py)
                        nc.sync.dma_start(out=y[j * TT:(j + 1) * TT, hs],
                                          in_=ys[:, hs])
                    return
                # c-outer so w2 chunks are consumed in arrival order
                for c in range(C_CH):
                    nc.tensor.matmul(py0, lhsT=at[:, c, :],
                                     rhs=w2_sb[:, c, 0:512],
                                     start=(c == 0), stop=(c == C_CH - 1))
                    nc.tensor.matmul(py1, lhsT=at[:, c, :],
                                     rhs=w2_sb[:, c, 512:1024],
                                     start=(c == 0), stop=(c == C_CH - 1))
                for h, py in ((0, py0), (1, py1)):
                    hs = slice(h * 512, (h + 1) * 512)
                    nc.vector.tensor_copy(ys[:, hs], py)
                    nc.sync.dma_start(out=y[j * TT:(j + 1) * TT, hs],
                                      in_=ys[:, hs])

            # pipeline: AB(0,1), A(2), B(0), A(3), B(1), ... A(7), B(5),
            # B(6), B(7)
            stage_ab01()
            for j in range(2, NTILES):
                stage_a(j)
                stage_b(j - 2)
            stage_b(NTILES - 2)
            stage_b(NTILES - 1, last=True)

    nc.finalize()
    return nc


def _get_program():
    global _PROGRAM
    if _PROGRAM is None:
        _PROGRAM = _build_program()
    return _PROGRAM


def _split_hi_lo_f16(a):
    hi = a.astype(np.float16)
    lo = (a - hi.astype(np.float32)).astype(np.float16)
    return hi, lo


def kernel(oldx, W_in, b_in, W_out):
    from concourse.bass_utils import run_bass_kernel_spmd

    oldx = np.asarray(oldx)
    W_in = np.asarray(W_in, dtype=np.float32)
    b_in = np.asarray(b_in, dtype=np.float32)
    W_out = np.asarray(W_out, dtype=np.float32)
    x = oldx.reshape(-1, DIM).astype(np.float32)          # [8192, 1024]

    # node-major column permutation: our col 8n+t  <-  ref col 255t+n
    i = np.arange(WIDTH)
    perm = 255 * (i % PAR) + (i // PAR)

    w1t = W_in[perm, :].T.astype(np.float32)              # [1024, 2040]
    w1t_hi = w1t.astype(np.float16)
    w1t_lo = ((w1t - w1t_hi.astype(np.float32)) * LO_SCALE).astype(np.float16)
    # [dim, width] -> [128, N_SLAB, K_CH, 512] with dim = k*128 + p,
    # width col = 512*slab + c (last slab zero-padded to 512)
    w1p = np.zeros((1024, N_SLAB * 512), np.float16)
    w1p[:, :WIDTH] = w1t_hi
    w1 = np.ascontiguousarray(
        w1p.reshape(K_CH, 128, N_SLAB, 512).transpose(1, 2, 0, 3))
    w1l = np.ascontiguousarray(
        w1t_lo.reshape(K_CH, 128, WIDTH).transpose(1, 0, 2)[:, :, :SH_COLS])
    b1p = b_in[perm].astype(np.float32)
    b1h = b1p.astype(np.float16)
    b1hl = np.ascontiguousarray(np.stack(
        [b1h, ((b1p - b1h.astype(np.float32)) * LO_SCALE).astype(np.float16)]))

    w2t = np.zeros((NODES_PAD, DIM), np.float32)
    w2t[:WIDTH] = W_out.T[perm, :]
    w2 = np.ascontiguousarray(
        w2t.astype(np.float16).reshape(C_CH, 128, DIM).transpose(1, 0, 2))
    ones2 = np.ascontiguousarray(np.stack(
        [np.full(128, 1.0, np.float16),
         np.full(128, 1.0 / LO_SCALE, np.float16)]))

    in_maps = []
    for c in range(N_CORES):
        xc = x[c * TOK_PER_CORE:(c + 1) * TOK_PER_CORE]   # [1024, 1024]
        xt_hi, xt_lo = _split_hi_lo_f16(xc.T)             # [dim, tok]
        # [dim, tok] -> [128, NTILES, K_CH, TT]; dim = k*128+p, tok = j*128+t
        xt_hi = xt_hi.reshape(K_CH, 128, NTILES, TT).transpose(1, 2, 0, 3)
        xt_lo = xt_lo.reshape(K_CH, 128, NTILES, TT).transpose(1, 2, 0, 3)
        xt = np.ascontiguousarray(np.stack([xt_hi, xt_lo], axis=2))
        in_maps.append({
            "xt": xt, "w1": w1, "w1l": w1l,
            "b1hl": b1hl, "w2": w2, "ones2": ones2,
        })

    nc = _get_program()
    res = run_bass_kernel_spmd(nc, in_maps, core_ids=list(range(N_CORES)))
    out = np.concatenate([res.results[c]["y"] for c in range(N_CORES)],
                         axis=0)
    return out.reshape(oldx.shape).astype(np.float32)


# revision 27
# speedup vs baseline: 1.1696x; 1.1675x over previous
"""Trainium2 Bass kernel for the FFF (fast feedforward / MoE-routing) module.

Math (per token x of dim 1024, PAR=8 trees of 255 nodes):
  logits = x @ W_in.T + b_in                      # [B, 2040]
  dec    = logits > 0
  acts   = silu(logits)
  dmap   = indicator of the 8 visited nodes per tree (root + 7 descents,
           descending by dec at the current node)
  out    = (acts * dmap) @ W_out.T                # [B, 1024]

Strategy (8 NeuronCores, data-parallel over the 8192 tokens, 1024 each):
  - GEMM1 in fp16 with region-dependent precision.  A decision flip at
    depth d corrupts 7-d downstream activations, so only the shallow
    nodes (0..15, levels 0..3) get the expensive treatment:
      cols   0..128 (nodes  0..15): x_hi*w + x_lo*w + x_hi*w_lo  (~fp32)
      cols 128..2040 (everything else): single x_hi*w pass
    The w_lo correction weights are pre-scaled by 2^10 (and x_hi by
    2^-10 on-device) so they stay in fp16 normal range.  fp32 bias is
    added on the vector engine.  Measured on the reference data this
    gives ~1.35e-2 overall rel err (gate is 2e-2).
  - dmap is built level-by-level with strided vector ops in a node-major
    column layout (col = 8*node + tree): child1 = V_d * dec_d (stride-2
    upsample), child0 = V_d - child1.
  - masked acts cast to fp16, transposed 128x128 on the PE, GEMM2 in fp16
    (exact products, fp32 PSUM accumulation).
  - startup: the 4.7MB of GEMM1 weights stream in as four 512-column
    slabs (one dma_start each; the DGE fair-shares ~300GB/s across
    in-flight dispatches, so fewer+ordered dispatches = earliest slab0).
    Tiles 0 and 1 are processed SLAB-MAJOR so each arriving slab feeds
    2 tiles of PE work; the fp32 bias arrives as fp16 (hi, 2^10*lo) rows
    and is broadcast across partitions on the PE (K=2 matmul against a
    (1, 2^-10) column pair -> exact fp32 in PSUM) during the initial
    weight wait instead of a 1MB broadcast DMA competing for early
    bandwidth.
"""

import numpy as np

DIM = 1024
PAR = 8
DEPTH = 7
N_NODES = 255
WIDTH = PAR * N_NODES          # 2040
NODES_PAD = 2048               # pad masked-acts/W_out^T to 16*128
N_CORES = 8
TOK_PER_CORE = 1024
TT = 128                       # tokens per tile
NTILES = TOK_PER_CORE // TT    # 8
K_CH = DIM // 128              # 8 contraction chunks for GEMM1
C_CH = NODES_PAD // 128        # 16 contraction chunks for GEMM2
DEC_COLS = 8 * 127             # 1016: decision nodes are levels 0..6
SH_COLS = 128                  # hi/lo-corrected region: nodes 0..15
LO_SCALE = 1024.0              # 2^10 keeps w_lo out of fp16 subnormals
N_SLAB = 4                     # w1 column slabs of 512 (last is 504+pad)

_PROGRAM = None


def _build_program():
    import concourse.bacc as bacc
    import concourse.tile as tile
    from concourse import mybir
    from concourse.masks import make_identity
    import concourse.bass as bass

    f32 = mybir.dt.float32
    f16 = mybir.dt.float16
    Alu = mybir.AluOpType
    Act = mybir.ActivationFunctionType

    nc = bacc.Bacc("TRN2", target_bir_lowering=False, debug=False,
                   num_devices=N_CORES)

    xt = nc.dram_tensor("xt", [128, NTILES, 2, K_CH, TT], f16,
                        kind="ExternalInput")
    w1 = nc.dram_tensor("w1", [128, N_SLAB, K_CH, 512], f16,
                        kind="ExternalInput")
    w1l = nc.dram_tensor("w1l", [128, K_CH, SH_COLS], f16,
                         kind="ExternalInput")
    b1hl = nc.dram_tensor("b1hl", [2, WIDTH], f16, kind="ExternalInput")
    ones2 = nc.dram_tensor("ones2", [2, 128], f16, kind="ExternalInput")
    w2 = nc.dram_tensor("w2", [128, C_CH, DIM], f16, kind="ExternalInput")
    y = nc.dram_tensor("y", [TOK_PER_CORE, DIM], f32, kind="ExternalOutput")

    SLAB_LIM = [(0, 512), (512, 1024), (1024, 1536), (1536, WIDTH)]

    with tile.TileContext(nc) as tc:
        with (
            tc.tile_pool(name="wts", bufs=1) as wts,
            tc.tile_pool(name="xts", bufs=4) as xts,
            tc.tile_pool(name="xh2s", bufs=2) as xh2s,
            tc.tile_pool(name="logits", bufs=2) as logits_pool,
            tc.tile_pool(name="mask", bufs=2) as mask_pool,
            tc.tile_pool(name="acts", bufs=2) as acts_pool,
            tc.tile_pool(name="mks", bufs=3) as mks_pool,
            tc.tile_pool(name="out", bufs=2) as out_pool,
            tc.tile_pool(name="pl", bufs=3, space="PSUM") as pl_pool,
            tc.tile_pool(name="pt", bufs=3, space="PSUM") as pt_pool,
            tc.tile_pool(name="py", bufs=2, space="PSUM") as py_pool,
        ):
            # ---- resident weights ----
            w1_sb = wts.tile([128, N_SLAB, K_CH, 512], f16)
            w1l_sb = wts.tile([128, K_CH, SH_COLS], f16)
            w2_sb = wts.tile([128, C_CH, DIM], f16)
            b1_sb = wts.tile([128, WIDTH], f32)
            b1_row = wts.tile([2, WIDTH], f16)
            ones = wts.tile([2, 128], f16)
            ident = wts.tile([128, 128], f16)

            xt_tiles = {}

            def prefetch_xt(j):
                xhl = xts.tile([128, 2, K_CH, TT], f16, tag="x")
                nc.sync.dma_start(out=xhl, in_=xt[:, j, :, :, :])
                xt_tiles[j] = xhl

            # Startup DMAs on the Sync engine in PE consumption order.
            # The DGE fair-shares bandwidth over in-flight dispatches and
            # completes them in dispatch order, so this order == arrival
            # order.
            nc.gpsimd.dma_start(out=ones, in_=ones2[:, :])
            nc.gpsimd.dma_start(out=b1_row, in_=b1hl[:, :])
            xhl0 = xts.tile([128, 2, K_CH, TT], f16, tag="x")
            xhl1 = xts.tile([128, 2, K_CH, TT], f16, tag="x")
            nc.sync.dma_start(out=xhl0[:, 0, 0:4], in_=xt[:, 0, 0, 0:4, :])
            nc.sync.dma_start(out=w1_sb[:, 0, 0:2], in_=w1[:, 0, 0:2])
            nc.sync.dma_start(out=xhl0[:, 0, 4:8], in_=xt[:, 0, 0, 4:8, :])
            nc.sync.dma_start(out=w1_sb[:, 0, 2:4], in_=w1[:, 0, 2:4])
            nc.sync.dma_start(out=xhl0[:, 1], in_=xt[:, 0, 1, :, :])
            nc.sync.dma_start(out=w1_sb[:, 0, 4:8], in_=w1[:, 0, 4:8])
            nc.sync.dma_start(out=w1l_sb, in_=w1l[:, :, :])
            nc.sync.dma_start(out=xhl1[:, 0], in_=xt[:, 1, 0, :, :])
            nc.sync.dma_start(out=xhl1[:, 1], in_=xt[:, 1, 1, :, :])
            xt_tiles[0] = xhl0
            xt_tiles[1] = xhl1
            for s in range(1, N_SLAB):
                nc.sync.dma_start(out=w1_sb[:, s], in_=w1[:, s])
            prefetch_xt(2)
            nc.sync.dma_start(out=w2_sb[:, 0:4, :], in_=w2[:, 0:4, :])
            prefetch_xt(3)
            nc.sync.dma_start(out=w2_sb[:, 4:8, :], in_=w2[:, 4:8, :])
            nc.sync.dma_start(out=w2_sb[:, 8:12, :], in_=w2[:, 8:12, :])
            nc.sync.dma_start(out=w2_sb[:, 12:16, :], in_=w2[:, 12:16, :])
            make_identity(nc, ident)

            # bias broadcast across partitions on the PE: fp16 (hi,
            # 2^10*lo) rows contracted (K=2) against (1, 2^-10) -> exact
            # fp32 in PSUM.  Emitted per-region inside stage_ab01, right
            # after each slab's first tile, so the PE's first instruction
            # is GEMM1 itself (gated only on x + slab0) and each bias mm
            # slots into a DMA gap.
            def bias_region(s):
                c0, c1 = SLAB_LIM[s]
                pb = pl_pool.tile([TT, 512], f32, tag="pl")
                nc.tensor.matmul(pb[:, 0:c1 - c0], lhsT=ones,
                                 rhs=b1_row[:, c0:c1], start=True, stop=True)
                nc.vector.tensor_copy(b1_sb[:, c0:c1], pb[:, 0:c1 - c0])

            state = {}

            def epilogue_vec(j, lg, d1, vv, ac):
                # tree mask: V_0 = 1 at root cols; then per level
                # child1 = V_d * dec_d, child0 = V_d - child1
                nc.vector.memset(vv[:, 0:8], 1.0)
                for d in range(DEPTH):
                    ld = 8 * (1 << d)
                    c0 = 8 * ((1 << d) - 1)
                    c1 = 8 * ((1 << (d + 1)) - 1)
                    vpar = vv[:, c0:c0 + ld].rearrange("p (i t) -> p i t", t=8)
                    dpar = d1[:, c0:c0 + ld].rearrange("p (i t) -> p i t", t=8)
                    kids = vv[:, c1:c1 + 2 * ld].rearrange(
                        "p (i two t) -> p i two t", two=2, t=8)
                    nc.vector.tensor_tensor(kids[:, :, 1, :], vpar, dpar,
                                            Alu.mult)
                    nc.vector.tensor_tensor(kids[:, :, 0, :], vpar,
                                            kids[:, :, 1, :], Alu.subtract)

            def finish_mask(j, ac, vv):
                mk = mks_pool.tile([TT, NODES_PAD], f16, tag="mk")
                nc.vector.memset(mk[:, WIDTH:NODES_PAD], 0.0)
                nc.vector.tensor_tensor(mk[:, 0:1024], ac[:, 0:1024],
                                        vv[:, 0:1024], Alu.mult)
                nc.vector.tensor_tensor(mk[:, 1024:WIDTH], ac[:, 1024:WIDTH],
                                        vv[:, 1024:WIDTH], Alu.mult)
                state[j] = mk

            def gemm1_slab_mm(s, xh, xl, xh2):
                c0, c1 = SLAB_LIM[s]
                w = c1 - c0
                p = pl_pool.tile([TT, 512], f32, tag="pl")
                if s == 0:
                    for k in range(K_CH):
                        nc.tensor.matmul(p, lhsT=xh[:, k, :],
                                         rhs=w1_sb[:, 0, k, :],
                                         start=(k == 0), stop=False)
                    for k in range(K_CH):
                        nc.tensor.matmul(p[:, 0:SH_COLS], lhsT=xl[:, k, :],
                                         rhs=w1_sb[:, 0, k, 0:SH_COLS],
                                         start=False, stop=False)
                    for k in range(K_CH):
                        nc.tensor.matmul(p[:, 0:SH_COLS], lhsT=xh2[:, k, :],
                                         rhs=w1l_sb[:, k, :],
                                         start=False, stop=(k == K_CH - 1))
                else:
                    for k in range(K_CH):
                        nc.tensor.matmul(p[:, 0:w], lhsT=xh[:, k, :],
                                         rhs=w1_sb[:, s, k, 0:w],
                                         start=(k == 0),
                                         stop=(k == K_CH - 1))
                return p

            def gemm1_slab_post(s, p, lg, d1, ac):
                c0, c1 = SLAB_LIM[s]
                w = c1 - c0
                nc.vector.tensor_tensor(lg[:, c0:c1], p[:, 0:w],
                                        b1_sb[:, c0:c1], Alu.add)
                if s == 0:
                    nc.vector.tensor_scalar(d1[:, 0:512], lg[:, 0:512], 0.0,
                                            None, Alu.is_gt)
                elif s == 1:
                    nc.vector.tensor_scalar(d1[:, 512:DEC_COLS],
                                            lg[:, 512:DEC_COLS], 0.0,
                                            None, Alu.is_gt)
                nc.scalar.activation(ac[:, c0:c1], lg[:, c0:c1], Act.Silu)

            def gemm1_slab(s, xh, xl, xh2, lg, d1, ac):
                p = gemm1_slab_mm(s, xh, xl, xh2)
                gemm1_slab_post(s, p, lg, d1, ac)

            def tile_bufs(j):
                xhl = xt_tiles.pop(j)
                xh, xl = xhl[:, 0], xhl[:, 1]
                xh2 = xh2s.tile([128, K_CH, TT], f16, tag="xh2")
                nc.vector.tensor_scalar(xh2, xh, 1.0 / LO_SCALE, None,
                                        Alu.mult)
                lg = logits_pool.tile([TT, WIDTH], f32, tag="lg")
                d1 = mask_pool.tile([TT, DEC_COLS], f16, tag="d1")
                vv = mask_pool.tile([TT, WIDTH], f16, tag="vv")
                ac = acts_pool.tile([TT, WIDTH], f16, tag="ac")
                return xh, xl, xh2, lg, d1, vv, ac

            def stage_a(j):
                if j + 1 < NTILES and j + 1 not in xt_tiles:
                    prefetch_xt(j + 1)
                xh, xl, xh2, lg, d1, vv, ac = tile_bufs(j)
                for s in range(N_SLAB):
                    gemm1_slab(s, xh, xl, xh2, lg, d1, ac)
                epilogue_vec(j, lg, d1, vv, ac)
                finish_mask(j, ac, vv)

            def stage_ab01():
                # tiles 0 and 1 slab-major: each arriving w1 slab feeds
                # 2 tiles of PE work, halving the DMA-bound startup.
                b0 = tile_bufs(0)
                b1_ = tile_bufs(1)
                for s in range(N_SLAB):
                    p0 = gemm1_slab_mm(s, b0[0], b0[1], b0[2])
                    bias_region(s)
                    gemm1_slab_post(s, p0, b0[3], b0[4], b0[6])
                    gemm1_slab(s, b1_[0], b1_[1], b1_[2], b1_[3], b1_[4],
                               b1_[6])
                    if s == 1:
                        epilogue_vec(0, b0[3], b0[4], b0[5], b0[6])
                        epilogue_vec(1, b1_[3], b1_[4], b1_[5], b1_[6])
                finish_mask(0, b0[6], b0[5])
                finish_mask(1, b1_[6], b1_[5])

            def stage_b(j, last=False):
                mk = state.pop(j)
                at = acts_pool.tile([128, C_CH, TT], f16, tag="at")
                c = 0
                for gsz in (1, 3, 4, 4, 4):
                    pt = pt_pool.tile([128, 512], f16)
                    for i in range(gsz):
                        nc.tensor.transpose(
                            pt[:, i * 128:(i + 1) * 128],
                            mk[:, (c + i) * 128:(c + i + 1) * 128], ident)
                    nc.scalar.copy(
                        at[:, c:c + gsz, :],
                        pt[:, :gsz * 128].rearrange("p (c t) -> p c t", t=TT))
                    c += gsz
                ys = out_pool.tile([TT, DIM], f32, tag="ys")
                py0 = py_pool.tile([TT, 512], f32, tag="py")
                py1 = py_pool.tile([TT, 512], f32, tag="py")
                if last:
                    # serialize the halves so the first store drains while
                    # the second half is still on the PE (shorter tail)
                    for h, py in ((0, py0), (1, py1)):
                        hs = slice(h * 512, (h + 1) * 512)
                        for c in range(C_CH):
                            nc.tensor.matmul(py, lhsT=at[:, c, :],
                                             rhs=w2_sb[:, c, hs],
                                             start=(c == 0),
                                             stop=(c == C_CH - 1))
                        nc.vector.tensor_copy(ys[:, hs], py)
                        nc.sync.dma_start(out=y[j * TT:(j + 1) * TT, hs],
                                          in_=ys[:, hs])
                    return
                # c-outer so w2 chunks are consumed in arrival order
                for c in range(C_CH):
                    nc.tensor.matmul(py0, lhsT=at[:, c, :],
                                     rhs=w2_sb[:, c, 0:512],
                                     start=(c == 0), stop=(c == C_CH - 1))
                    nc.tensor.matmul(py1, lhsT=at[:, c, :],
                                     rhs=w2_sb[:, c, 512:1024],
                                     start=(c == 0), stop=(c == C_CH - 1))
                for h, py in ((0, py0), (1, py1)):
                    hs = slice(h * 512, (h + 1) * 512)
                    nc.vector.tensor_copy(ys[:, hs], py)
                    nc.sync.dma_start(out=y[j * TT:(j + 1) * TT, hs],
                                      in_=ys[:, hs])

            # pipeline: AB(0,1), A(2), B(0), A(3), B(1), ... A(7), B(5),
            # B(6), B(7)
            stage_ab01()
            for j in range(2, NTILES):
                stage_a(j)
                stage_b(j - 2)
            stage_b(NTILES - 2)
            stage_b(NTILES - 1, last=True)

    nc.finalize()
    return nc


def _get_program():
    global _PROGRAM
    if _PROGRAM is None:
        _PROGRAM = _build_program()
    return _PROGRAM


def _split_hi_lo_f16(a):
    hi = a.astype(np.float16)
    lo = (a - hi.astype(np.float32)).astype(np.float16)
    return hi, lo


def kernel(oldx, W_in, b_in, W_out):
    from concourse.bass_utils import run_bass_kernel_spmd

    oldx = np.asarray(oldx)
    W_in = np.asarray(W_in, dtype=np.float32)
    b_in = np.asarray(b_in, dtype=np.float32)
    W_out = np.asarray(W_out, dtype=np.float32)
    x = oldx.reshape(-1, DIM).astype(np.float32)          # [8192, 1024]

    # node-major column permutation: our col 8n+t  <-  ref col 255t+n
    i = np.arange(WIDTH)
    perm = 255 * (i % PAR) + (i // PAR)

    w1t = W_in[perm, :].T.astype(np.float32)              # [1024, 2040]
    w1t_hi = w1t.astype(np.float16)
    w1t_lo = ((w1t - w1t_hi.astype(np.float32)) * LO_SCALE).astype(np.float16)
    # [dim, width] -> [128, N_SLAB, K_CH, 512] with dim = k*128 + p,
    # width col = 512*slab + c (last slab zero-padded to 512)
    w1p = np.zeros((1024, N_SLAB * 512), np.float16)
    w1p[:, :WIDTH] = w1t_hi
    w1 = np.ascontiguousarray(
        w1p.reshape(K_CH, 128, N_SLAB, 512).transpose(1, 2, 0, 3))
    w1l = np.ascontiguousarray(
        w1t_lo.reshape(K_CH, 128, WIDTH).transpose(1, 0, 2)[:, :, :SH_COLS])
    b1p = b_in[perm].astype(np.float32)
    b1h = b1p.astype(np.float16)
    b1hl = np.ascontiguousarray(np.stack(
        [b1h, ((b1p - b1h.astype(np.float32)) * LO_SCALE).astype(np.float16)]))

    w2t = np.zeros((NODES_PAD, DIM), np.float32)
    w2t[:WIDTH] = W_out.T[perm, :]
    w2 = np.ascontiguousarray(
        w2t.astype(np.float16).reshape(C_CH, 128, DIM).transpose(1, 0, 2))
    ones2 = np.ascontiguousarray(np.stack(
        [np.full(128, 1.0, np.float16),
         np.full(128, 1.0 / LO_SCALE, np.float16)]))

    in_maps = []
    for c in range(N_CORES):
        xc = x[c * TOK_PER_CORE:(c + 1) * TOK_PER_CORE]   # [1024, 1024]
        xt_hi, xt_lo = _split_hi_lo_f16(xc.T)             # [dim, tok]
        # [dim, tok] -> [128, NTILES, K_CH, TT]; dim = k*128+p, tok = j*128+t
        xt_hi = xt_hi.reshape(K_CH, 128, NTILES, TT).transpose(1, 2, 0, 3)
        xt_lo = xt_lo.reshape(K_CH, 128, NTILES, TT).transpose(1, 2, 0, 3)
        xt = np.ascontiguousarray(np.stack([xt_hi, xt_lo], axis=2))
        in_maps.append({
            "xt": xt, "w1": w1, "w1l": w1l,
            "b1hl": b1hl, "w2": w2, "ones2": ones2,
        })

    nc = _get_program()
    res = run_bass_kernel_spmd(nc, in_maps, core_ids=list(range(N_CORES)))
    out = np.concatenate([res.results[c]["y"] for c in range(N_CORES)],
                         axis=0)
    return out.reshape(oldx.shape).astype(np.float32)


# revision 28
# speedup vs baseline: 1.1810x; 1.0097x over previous
"""Trainium2 Bass kernel for the FFF (fast feedforward / MoE-routing) module.

Math (per token x of dim 1024, PAR=8 trees of 255 nodes):
  logits = x @ W_in.T + b_in                      # [B, 2040]
  dec    = logits > 0
  acts   = silu(logits)
  dmap   = indicator of the 8 visited nodes per tree (root + 7 descents,
           descending by dec at the current node)
  out    = (acts * dmap) @ W_out.T                # [B, 1024]

Strategy (8 NeuronCores, data-parallel over the 8192 tokens, 1024 each):
  - GEMM1 in fp16 with region-dependent precision.  A decision flip at
    depth d corrupts 7-d downstream activations, so only the shallow
    nodes (0..15, levels 0..3) get the expensive treatment:
      cols   0..128 (nodes  0..15): x_hi*w + x_lo*w + x_hi*w_lo  (~fp32)
      cols 128..2040 (everything else): single x_hi*w pass
    The w_lo correction weights are pre-scaled by 2^10 (and x_hi by
    2^-10 on-device) so they stay in fp16 normal range.  fp32 bias is
    added on the vector engine.  Measured on the reference data this
    gives ~1.35e-2 overall rel err (gate is 2e-2).
  - dmap is built level-by-level with strided vector ops in a node-major
    column layout (col = 8*node + tree): child1 = V_d * dec_d (stride-2
    upsample), child0 = V_d - child1.
  - masked acts cast to fp16, transposed 128x128 on the PE, GEMM2 in fp16
    (exact products, fp32 PSUM accumulation).
  - startup: the 4.7MB of GEMM1 weights stream in as four 512-column
    slabs (one dma_start each; the DGE fair-shares ~300GB/s across
    in-flight dispatches, so fewer+ordered dispatches = earliest slab0).
    Tiles 0 and 1 are processed SLAB-MAJOR so each arriving slab feeds
    2 tiles of PE work; the fp32 bias arrives as fp16 (hi, 2^10*lo) rows
    and is broadcast across partitions on the PE (K=2 matmul against a
    (1, 2^-10) column pair -> exact fp32 in PSUM) during the initial
    weight wait instead of a 1MB broadcast DMA competing for early
    bandwidth.
"""

import numpy as np

DIM = 1024
PAR = 8
DEPTH = 7
N_NODES = 255
WIDTH = PAR * N_NODES          # 2040
NODES_PAD = 2048               # pad masked-acts/W_out^T to 16*128
N_CORES = 8
TOK_PER_CORE = 1024
TT = 128                       # tokens per tile
NTILES = TOK_PER_CORE // TT    # 8
K_CH = DIM // 128              # 8 contraction chunks for GEMM1
C_CH = NODES_PAD // 128        # 16 contraction chunks for GEMM2
DEC_COLS = 8 * 127             # 1016: decision nodes are levels 0..6
SH_COLS = 128                  # hi/lo-corrected region: nodes 0..15
LO_SCALE = 1024.0              # 2^10 keeps w_lo out of fp16 subnormals
N_SLAB = 4                     # w1 column slabs of 512 (last is 504+pad)

_PROGRAM = None


def _build_program():
    import concourse.bacc as bacc
    import concourse.tile as tile
    from concourse import mybir
    from concourse.masks import make_identity
    import concourse.bass as bass

    f32 = mybir.dt.float32
    f16 = mybir.dt.float16
    Alu = mybir.AluOpType
    Act = mybir.ActivationFunctionType

    nc = bacc.Bacc("TRN2", target_bir_lowering=False, debug=False,
                   num_devices=N_CORES)

    xt = nc.dram_tensor("xt", [128, NTILES, 2, K_CH, TT], f16,
                        kind="ExternalInput")
    w1 = nc.dram_tensor("w1", [128, N_SLAB, K_CH, 512], f16,
                        kind="ExternalInput")
    w1l = nc.dram_tensor("w1l", [128, K_CH, SH_COLS], f16,
                         kind="ExternalInput")
    b1hl = nc.dram_tensor("b1hl", [2, WIDTH], f16, kind="ExternalInput")
    ones2 = nc.dram_tensor("ones2", [2, 128], f16, kind="ExternalInput")
    w2 = nc.dram_tensor("w2", [128, C_CH, DIM], f16, kind="ExternalInput")
    y = nc.dram_tensor("y", [TOK_PER_CORE, DIM], f32, kind="ExternalOutput")

    SLAB_LIM = [(0, 512), (512, 1024), (1024, 1536), (1536, WIDTH)]

    with tile.TileContext(nc) as tc:
        with (
            tc.tile_pool(name="wts", bufs=1) as wts,
            tc.tile_pool(name="xts", bufs=4) as xts,
            tc.tile_pool(name="xh2s", bufs=2) as xh2s,
            tc.tile_pool(name="logits", bufs=2) as logits_pool,
            tc.tile_pool(name="mask", bufs=2) as mask_pool,
            tc.tile_pool(name="acts", bufs=2) as acts_pool,
            tc.tile_pool(name="mks", bufs=3) as mks_pool,
            tc.tile_pool(name="out", bufs=2) as out_pool,
            tc.tile_pool(name="pl", bufs=3, space="PSUM") as pl_pool,
            tc.tile_pool(name="pt", bufs=3, space="PSUM") as pt_pool,
            tc.tile_pool(name="py", bufs=2, space="PSUM") as py_pool,
        ):
            # ---- resident weights ----
            w1_sb = wts.tile([128, N_SLAB, K_CH, 512], f16)
            w1l_sb = wts.tile([128, K_CH, SH_COLS], f16)
            w2_sb = wts.tile([128, C_CH, DIM], f16)
            b1_sb = wts.tile([128, WIDTH], f32)
            b1_row = wts.tile([2, WIDTH], f16)
            ones = wts.tile([2, 128], f16)
            ident = wts.tile([128, 128], f16)

            xt_tiles = {}

            def prefetch_xt(j):
                xhl = xts.tile([128, 2, K_CH, TT], f16, tag="x")
                nc.sync.dma_start(out=xhl, in_=xt[:, j, :, :, :])
                xt_tiles[j] = xhl

            # Startup DMAs on the Sync engine in PE consumption order.
            # The DGE fair-shares bandwidth over in-flight dispatches and
            # completes them in dispatch order, so this order == arrival
            # order.
            nc.gpsimd.dma_start(out=ones, in_=ones2[:, :])
            nc.gpsimd.dma_start(out=b1_row, in_=b1hl[:, :])
            xhl0 = xts.tile([128, 2, K_CH, TT], f16, tag="x")
            xhl1 = xts.tile([128, 2, K_CH, TT], f16, tag="x")
            nc.sync.dma_start(out=xhl0[:, 0], in_=xt[:, 0, 0, :, :])
            nc.sync.dma_start(out=w1_sb[:, 0, 0:4], in_=w1[:, 0, 0:4])
            nc.sync.dma_start(out=xhl0[:, 1], in_=xt[:, 0, 1, :, :])
            nc.sync.dma_start(out=w1_sb[:, 0, 4:8], in_=w1[:, 0, 4:8])
            nc.sync.dma_start(out=w1l_sb, in_=w1l[:, :, :])
            nc.sync.dma_start(out=xhl1[:, 0], in_=xt[:, 1, 0, :, :])
            nc.sync.dma_start(out=xhl1[:, 1], in_=xt[:, 1, 1, :, :])
            xt_tiles[0] = xhl0
            xt_tiles[1] = xhl1
            for s in range(1, N_SLAB):
                nc.sync.dma_start(out=w1_sb[:, s], in_=w1[:, s])
            prefetch_xt(2)
            nc.sync.dma_start(out=w2_sb[:, 0:4, :], in_=w2[:, 0:4, :])
            prefetch_xt(3)
            nc.sync.dma_start(out=w2_sb[:, 4:8, :], in_=w2[:, 4:8, :])
            nc.sync.dma_start(out=w2_sb[:, 8:12, :], in_=w2[:, 8:12, :])
            nc.sync.dma_start(out=w2_sb[:, 12:16, :], in_=w2[:, 12:16, :])
            make_identity(nc, ident)

            # bias broadcast across partitions on the PE: fp16 (hi,
            # 2^10*lo) rows contracted (K=2) against (1, 2^-10) -> exact
            # fp32 in PSUM.  Emitted per-region inside stage_ab01, right
            # after each slab's first tile, so the PE's first instruction
            # is GEMM1 itself (gated only on x + slab0) and each bias mm
            # slots into a DMA gap.
            def bias_region(s):
                c0, c1 = SLAB_LIM[s]
                pb = pl_pool.tile([TT, 512], f32, tag="pl")
                nc.tensor.matmul(pb[:, 0:c1 - c0], lhsT=ones,
                                 rhs=b1_row[:, c0:c1], start=True, stop=True)
                nc.vector.tensor_copy(b1_sb[:, c0:c1], pb[:, 0:c1 - c0])

            state = {}

            def epilogue_vec(j, lg, d1, vv, ac):
                # tree mask: V_0 = 1 at root cols; then per level
                # child1 = V_d * dec_d, child0 = V_d - child1
                nc.vector.memset(vv[:, 0:8], 1.0)
                for d in range(DEPTH):
                    ld = 8 * (1 << d)
                    c0 = 8 * ((1 << d) - 1)
                    c1 = 8 * ((1 << (d + 1)) - 1)
                    vpar = vv[:, c0:c0 + ld].rearrange("p (i t) -> p i t", t=8)
                    dpar = d1[:, c0:c0 + ld].rearrange("p (i t) -> p i t", t=8)
                    kids = vv[:, c1:c1 + 2 * ld].rearrange(
                        "p (i two t) -> p i two t", two=2, t=8)
                    nc.vector.tensor_tensor(kids[:, :, 1, :], vpar, dpar,
                                            Alu.mult)
                    nc.vector.tensor_tensor(kids[:, :, 0, :], vpar,
                                            kids[:, :, 1, :], Alu.subtract)

            def finish_mask(j, ac, vv):
                mk = mks_pool.tile([TT, NODES_PAD], f16, tag="mk")
                nc.vector.memset(mk[:, WIDTH:NODES_PAD], 0.0)
                nc.vector.tensor_tensor(mk[:, 0:1024], ac[:, 0:1024],
                                        vv[:, 0:1024], Alu.mult)
                nc.vector.tensor_tensor(mk[:, 1024:WIDTH], ac[:, 1024:WIDTH],
                                        vv[:, 1024:WIDTH], Alu.mult)
                state[j] = mk

            def gemm1_slab_mm(s, xh, xl, xh2):
                c0, c1 = SLAB_LIM[s]
                w = c1 - c0
                p = pl_pool.tile([TT, 512], f32, tag="pl")
                if s == 0:
                    for k in range(K_CH):
                        nc.tensor.matmul(p, lhsT=xh[:, k, :],
                                         rhs=w1_sb[:, 0, k, :],
                                         start=(k == 0), stop=False)
                    for k in range(K_CH):
                        nc.tensor.matmul(p[:, 0:SH_COLS], lhsT=xl[:, k, :],
                                         rhs=w1_sb[:, 0, k, 0:SH_COLS],
                                         start=False, stop=False)
                    for k in range(K_CH):
                        nc.tensor.matmul(p[:, 0:SH_COLS], lhsT=xh2[:, k, :],
                                         rhs=w1l_sb[:, k, :],
                                         start=False, stop=(k == K_CH - 1))
                else:
                    for k in range(K_CH):
                        nc.tensor.matmul(p[:, 0:w], lhsT=xh[:, k, :],
                                         rhs=w1_sb[:, s, k, 0:w],
                                         start=(k == 0),
                                         stop=(k == K_CH - 1))
                return p

            def gemm1_slab_post(s, p, lg, d1, ac):
                c0, c1 = SLAB_LIM[s]
                w = c1 - c0
                nc.vector.tensor_tensor(lg[:, c0:c1], p[:, 0:w],
                                        b1_sb[:, c0:c1], Alu.add)
                if s == 0:
                    nc.vector.tensor_scalar(d1[:, 0:512], lg[:, 0:512], 0.0,
                                            None, Alu.is_gt)
                elif s == 1:
                    nc.vector.tensor_scalar(d1[:, 512:DEC_COLS],
                                            lg[:, 512:DEC_COLS], 0.0,
                                            None, Alu.is_gt)
                nc.scalar.activation(ac[:, c0:c1], lg[:, c0:c1], Act.Silu)

            def gemm1_slab(s, xh, xl, xh2, lg, d1, ac):
                p = gemm1_slab_mm(s, xh, xl, xh2)
                gemm1_slab_post(s, p, lg, d1, ac)

            def tile_bufs(j):
                xhl = xt_tiles.pop(j)
                xh, xl = xhl[:, 0], xhl[:, 1]
                xh2 = xh2s.tile([128, K_CH, TT], f16, tag="xh2")
                nc.vector.tensor_scalar(xh2, xh, 1.0 / LO_SCALE, None,
                                        Alu.mult)
                lg = logits_pool.tile([TT, WIDTH], f32, tag="lg")
                d1 = mask_pool.tile([TT, DEC_COLS], f16, tag="d1")
                vv = mask_pool.tile([TT, WIDTH], f16, tag="vv")
                ac = acts_pool.tile([TT, WIDTH], f16, tag="ac")
                return xh, xl, xh2, lg, d1, vv, ac

            def stage_a(j):
                if j + 1 < NTILES and j + 1 not in xt_tiles:
                    prefetch_xt(j + 1)
                xh, xl, xh2, lg, d1, vv, ac = tile_bufs(j)
                for s in range(N_SLAB):
                    gemm1_slab(s, xh, xl, xh2, lg, d1, ac)
                epilogue_vec(j, lg, d1, vv, ac)
                finish_mask(j, ac, vv)

            def stage_ab01():
                # tiles 0 and 1 slab-major: each arriving w1 slab feeds
                # 2 tiles of PE work, halving the DMA-bound startup.
                b0 = tile_bufs(0)
                b1_ = tile_bufs(1)
                for s in range(N_SLAB):
                    p0 = gemm1_slab_mm(s, b0[0], b0[1], b0[2])
                    bias_region(s)
                    gemm1_slab_post(s, p0, b0[3], b0[4], b0[6])
                    gemm1_slab(s, b1_[0], b1_[1], b1_[2], b1_[3], b1_[4],
                               b1_[6])
                    if s == 1:
                        epilogue_vec(0, b0[3], b0[4], b0[5], b0[6])
                        epilogue_vec(1, b1_[3], b1_[4], b1_[5], b1_[6])
                finish_mask(0, b0[6], b0[5])
                finish_mask(1, b1_[6], b1_[5])

            def stage_b(j, last=False):
                mk = state.pop(j)
                at = acts_pool.tile([128, C_CH, TT], f16, tag="at")
                c = 0
                for gsz in (1, 3, 4, 4, 4):
                    pt = pt_pool.tile([128, 512], f16)
                    for i in range(gsz):
                        nc.tensor.transpose(
                            pt[:, i * 128:(i + 1) * 128],
                            mk[:, (c + i) * 128:(c + i + 1) * 128], ident)
                    nc.scalar.copy(
                        at[:, c:c + gsz, :],
                        pt[:, :gsz * 128].rearrange("p (c t) -> p c t", t=TT))
                    c += gsz
                ys = out_pool.tile([TT, DIM], f32, tag="ys")
                py0 = py_pool.tile([TT, 512], f32, tag="py")
                py1 = py_pool.tile([TT, 512], f32, tag="py")
                if last:
                    # serialize the halves so the first store drains while
                    # the second half is still on the PE (shorter tail)
                    for h, py in ((0, py0), (1, py1)):
                        hs = slice(h * 512, (h + 1) * 512)
                        for c in range(C_CH):
                            nc.tensor.matmul(py, lhsT=at[:, c, :],
                                             rhs=w2_sb[:, c, hs],
                                             start=(c == 0),
                                             stop=(c == C_CH - 1))
                        nc.vector.tensor_copy(ys[:, hs], py)
                        nc.sync.dma_start(out=y[j * TT:(j + 1) * TT, hs],
                                          in_=ys[:, hs])
                    return
                # c-outer so w2 chunks are consumed in arrival order
                for c in range(C_CH):
                    nc.tensor.matmul(py0, lhsT=at[:, c, :],
                                     rhs=w2_sb[:, c, 0:512],
                                     start=(c == 0), stop=(c == C_CH - 1))
                    nc.tensor.matmul(py1, lhsT=at[:, c, :],
                                     rhs=w2_sb[:, c, 512:1024],
                                     start=(c == 0), stop=(c == C_CH - 1))
                for h, py in ((0, py0), (1, py1)):
                    hs = slice(h * 512, (h + 1) * 512)
                    nc.vector.tensor_copy(ys[:, hs], py)
                    nc.sync.dma_start(out=y[j * TT:(j + 1) * TT, hs],
                                      in_=ys[:, hs])

            # pipeline: AB(0,1), A(2), B(0), A(3), B(1), ... A(7), B(5),
            # B(6), B(7)
            stage_ab01()
            for j in range(2, NTILES):
                stage_a(j)
                stage_b(j - 2)
            stage_b(NTILES - 2)
            stage_b(NTILES - 1, last=True)

    nc.finalize()
    return nc


def _get_program():
    global _PROGRAM
    if _PROGRAM is None:
        _PROGRAM = _build_program()
    return _PROGRAM


def _split_hi_lo_f16(a):
    hi = a.astype(np.float16)
    lo = (a - hi.astype(np.float32)).astype(np.float16)
    return hi, lo


def kernel(oldx, W_in, b_in, W_out):
    from concourse.bass_utils import run_bass_kernel_spmd

    oldx = np.asarray(oldx)
    W_in = np.asarray(W_in, dtype=np.float32)
    b_in = np.asarray(b_in, dtype=np.float32)
    W_out = np.asarray(W_out, dtype=np.float32)
    x = oldx.reshape(-1, DIM).astype(np.float32)          # [8192, 1024]

    # node-major column permutation: our col 8n+t  <-  ref col 255t+n
    i = np.arange(WIDTH)
    perm = 255 * (i % PAR) + (i // PAR)

    w1t = W_in[perm, :].T.astype(np.float32)              # [1024, 2040]
    w1t_hi = w1t.astype(np.float16)
    w1t_lo = ((w1t - w1t_hi.astype(np.float32)) * LO_SCALE).astype(np.float16)
    # [dim, width] -> [128, N_SLAB, K_CH, 512] with dim = k*128 + p,
    # width col = 512*slab + c (last slab zero-padded to 512)
    w1p = np.zeros((1024, N_SLAB * 512), np.float16)
    w1p[:, :WIDTH] = w1t_hi
    w1 = np.ascontiguousarray(
        w1p.reshape(K_CH, 128, N_SLAB, 512).transpose(1, 2, 0, 3))
    w1l = np.ascontiguousarray(
        w1t_lo.reshape(K_CH, 128, WIDTH).transpose(1, 0, 2)[:, :, :SH_COLS])
    b1p = b_in[perm].astype(np.float32)
    b1h = b1p.astype(np.float16)
    b1hl = np.ascontiguousarray(np.stack(
        [b1h, ((b1p - b1h.astype(np.float32)) * LO_SCALE).astype(np.float16)]))

    w2t = np.zeros((NODES_PAD, DIM), np.float32)
    w2t[:WIDTH] = W_out.T[perm, :]
    w2 = np.ascontiguousarray(
        w2t.astype(np.float16).reshape(C_CH, 128, DIM).transpose(1, 0, 2))
    ones2 = np.ascontiguousarray(np.stack(
        [np.full(128, 1.0, np.float16),
         np.full(128, 1.0 / LO_SCALE, np.float16)]))

    in_maps = []
    for c in range(N_CORES):
        xc = x[c * TOK_PER_CORE:(c + 1) * TOK_PER_CORE]   # [1024, 1024]
        xt_hi, xt_lo = _split_hi_lo_f16(xc.T)             # [dim, tok]
        # [dim, tok] -> [128, NTILES, K_CH, TT]; dim = k*128+p, tok = j*128+t
        xt_hi = xt_hi.reshape(K_CH, 128, NTILES, TT).transpose(1, 2, 0, 3)
        xt_lo = xt_lo.reshape(K_CH, 128, NTILES, TT).transpose(1, 2, 0, 3)
        xt = np.ascontiguousarray(np.stack([xt_hi, xt_lo], axis=2))
        in_maps.append({
            "xt": xt, "w1": w1, "w1l": w1l,
            "b1hl": b1hl, "w2": w2, "ones2": ones2,
        })

    nc = _get_program()
    res = run_bass_kernel_spmd(nc, in_maps, core_ids=list(range(N_CORES)))
    out = np.concatenate([res.results[c]["y"] for c in range(N_CORES)],
                         axis=0)
    return out.reshape(oldx.shape).astype(np.float32)


# revision 29
# speedup vs baseline: 1.1814x; 1.0003x over previous
"""Trainium2 Bass kernel for the FFF (fast feedforward / MoE-routing) module.

Math (per token x of dim 1024, PAR=8 trees of 255 nodes):
  logits = x @ W_in.T + b_in                      # [B, 2040]
  dec    = logits > 0
  acts   = silu(logits)
  dmap   = indicator of the 8 visited nodes per tree (root + 7 descents,
           descending by dec at the current node)
  out    = (acts * dmap) @ W_out.T                # [B, 1024]

Strategy (8 NeuronCores, data-parallel over the 8192 tokens, 1024 each):
  - GEMM1 in fp16 with region-dependent precision.  A decision flip at
    depth d corrupts 7-d downstream activations, so only the shallow
    nodes (0..15, levels 0..3) get the expensive treatment:
      cols   0..128 (nodes  0..15): x_hi*w + x_lo*w + x_hi*w_lo  (~fp32)
      cols 128..2040 (everything else): single x_hi*w pass
    The w_lo correction weights are pre-scaled by 2^10 (and x_hi by
    2^-10 on-device) so they stay in fp16 normal range.  fp32 bias is
    added on the vector engine.  Measured on the reference data this
    gives ~1.35e-2 overall rel err (gate is 2e-2).
  - dmap is built level-by-level with strided vector ops in a node-major
    column layout (col = 8*node + tree): child1 = V_d * dec_d (stride-2
    upsample), child0 = V_d - child1.
  - masked acts cast to fp16, transposed 128x128 on the PE, GEMM2 in fp16
    (exact products, fp32 PSUM accumulation).
  - startup: the 4.7MB of GEMM1 weights stream in as four 512-column
    slabs (one dma_start each; the DGE fair-shares ~300GB/s across
    in-flight dispatches, so fewer+ordered dispatches = earliest slab0).
    Tiles 0 and 1 are processed SLAB-MAJOR so each arriving slab feeds
    2 tiles of PE work; the fp32 bias arrives as fp16 (hi, 2^10*lo) rows
    and is broadcast across partitions on the PE (K=2 matmul against a
    (1, 2^-10) column pair -> exact fp32 in PSUM) during the initial
    weight wait instead of a 1MB broadcast DMA competing for early
    bandwidth.
"""

import numpy as np

DIM = 1024
PAR = 8
DEPTH = 7
N_NODES = 255
WIDTH = PAR * N_NODES          # 2040
NODES_PAD = 2048               # pad masked-acts/W_out^T to 16*128
N_CORES = 8
TOK_PER_CORE = 1024
TT = 128                       # tokens per tile
NTILES = TOK_PER_CORE // TT    # 8
K_CH = DIM // 128              # 8 contraction chunks for GEMM1
C_CH = NODES_PAD // 128        # 16 contraction chunks for GEMM2
DEC_COLS = 8 * 127             # 1016: decision nodes are levels 0..6
SH_COLS = 128                  # hi/lo-corrected region: nodes 0..15
LO_SCALE = 1024.0              # 2^10 keeps w_lo out of fp16 subnormals
N_SLAB = 4                     # w1 column slabs of 512 (last is 504+pad)

_PROGRAM = None


def _build_program():
    import concourse.bacc as bacc
    import concourse.tile as tile
    from concourse import mybir
    from concourse.masks import make_identity
    import concourse.bass as bass

    f32 = mybir.dt.float32
    f16 = mybir.dt.float16
    Alu = mybir.AluOpType
    Act = mybir.ActivationFunctionType

    nc = bacc.Bacc("TRN2", target_bir_lowering=False, debug=False,
                   num_devices=N_CORES)

    xt = nc.dram_tensor("xt", [128, NTILES, 2, K_CH, TT], f16,
                        kind="ExternalInput")
    w1 = nc.dram_tensor("w1", [128, N_SLAB, K_CH, 512], f16,
                        kind="ExternalInput")
    w1l = nc.dram_tensor("w1l", [128, K_CH, SH_COLS], f16,
                         kind="ExternalInput")
    b1hl = nc.dram_tensor("b1hl", [2, WIDTH], f16, kind="ExternalInput")
    ones2 = nc.dram_tensor("ones2", [2, 128], f16, kind="ExternalInput")
    w2 = nc.dram_tensor("w2", [128, C_CH, DIM], f16, kind="ExternalInput")
    y = nc.dram_tensor("y", [TOK_PER_CORE, DIM], f32, kind="ExternalOutput")

    SLAB_LIM = [(0, 512), (512, 1024), (1024, 1536), (1536, WIDTH)]

    with tile.TileContext(nc) as tc:
        with (
            tc.tile_pool(name="wts", bufs=1) as wts,
            tc.tile_pool(name="xts", bufs=4) as xts,
            tc.tile_pool(name="xh2s", bufs=2) as xh2s,
            tc.tile_pool(name="logits", bufs=2) as logits_pool,
            tc.tile_pool(name="mask", bufs=2) as mask_pool,
            tc.tile_pool(name="acts", bufs=2) as acts_pool,
            tc.tile_pool(name="mks", bufs=3) as mks_pool,
            tc.tile_pool(name="out", bufs=2) as out_pool,
            tc.tile_pool(name="pl", bufs=3, space="PSUM") as pl_pool,
            tc.tile_pool(name="pt", bufs=3, space="PSUM") as pt_pool,
            tc.tile_pool(name="py", bufs=2, space="PSUM") as py_pool,
        ):
            # ---- resident weights ----
            w1_sb = wts.tile([128, N_SLAB, K_CH, 512], f16)
            w1l_sb = wts.tile([128, K_CH, SH_COLS], f16)
            w2_sb = wts.tile([128, C_CH, DIM], f16)
            b1_sb = wts.tile([128, WIDTH], f32)
            b1_row = wts.tile([2, WIDTH], f16)
            ones = wts.tile([2, 128], f16)
            ident = wts.tile([128, 128], f16)

            xt_tiles = {}

            def prefetch_xt(j):
                xhl = xts.tile([128, 2, K_CH, TT], f16, tag="x")
                nc.sync.dma_start(out=xhl, in_=xt[:, j, :, :, :])
                xt_tiles[j] = xhl

            # Startup DMAs on the Sync engine in PE consumption order.
            # The DGE fair-shares bandwidth over in-flight dispatches and
            # completes them in dispatch order, so this order == arrival
            # order.
            nc.gpsimd.dma_start(out=ones, in_=ones2[:, :])
            nc.gpsimd.dma_start(out=b1_row, in_=b1hl[:, :])
            xhl0 = xts.tile([128, 2, K_CH, TT], f16, tag="x")
            xhl1 = xts.tile([128, 2, K_CH, TT], f16, tag="x")
            nc.sync.dma_start(out=xhl0[:, 0], in_=xt[:, 0, 0, :, :])
            nc.sync.dma_start(out=w1_sb[:, 0, 0:4], in_=w1[:, 0, 0:4])
            nc.sync.dma_start(out=xhl0[:, 1], in_=xt[:, 0, 1, :, :])
            nc.sync.dma_start(out=w1_sb[:, 0, 4:8], in_=w1[:, 0, 4:8])
            nc.sync.dma_start(out=w1l_sb, in_=w1l[:, :, :])
            nc.sync.dma_start(out=xhl1[:, 0], in_=xt[:, 1, 0, :, :])
            nc.sync.dma_start(out=xhl1[:, 1], in_=xt[:, 1, 1, :, :])
            xt_tiles[0] = xhl0
            xt_tiles[1] = xhl1
            for s in range(1, N_SLAB):
                nc.sync.dma_start(out=w1_sb[:, s], in_=w1[:, s])
            prefetch_xt(2)
            nc.sync.dma_start(out=w2_sb[:, 0:4, :], in_=w2[:, 0:4, :])
            prefetch_xt(3)
            nc.sync.dma_start(out=w2_sb[:, 4:8, :], in_=w2[:, 4:8, :])
            nc.sync.dma_start(out=w2_sb[:, 8:12, :], in_=w2[:, 8:12, :])
            nc.sync.dma_start(out=w2_sb[:, 12:16, :], in_=w2[:, 12:16, :])
            make_identity(nc, ident)

            # bias broadcast across partitions on the PE: fp16 (hi,
            # 2^10*lo) rows contracted (K=2) against (1, 2^-10) -> exact
            # fp32 in PSUM.  Emitted per-region inside stage_ab01, right
            # after each slab's first tile, so the PE's first instruction
            # is GEMM1 itself (gated only on x + slab0) and each bias mm
            # slots into a DMA gap.
            def bias_region(s):
                c0, c1 = SLAB_LIM[s]
                pb = pl_pool.tile([TT, 512], f32, tag="pl")
                nc.tensor.matmul(pb[:, 0:c1 - c0], lhsT=ones,
                                 rhs=b1_row[:, c0:c1], start=True, stop=True)
                nc.vector.tensor_copy(b1_sb[:, c0:c1], pb[:, 0:c1 - c0])

            state = {}

            def epilogue_vec(j, lg, d1, vv, ac):
                # tree mask: V_0 = 1 at root cols; then per level
                # child1 = V_d * dec_d, child0 = V_d - child1
                nc.vector.memset(vv[:, 0:8], 1.0)
                for d in range(DEPTH):
                    ld = 8 * (1 << d)
                    c0 = 8 * ((1 << d) - 1)
                    c1 = 8 * ((1 << (d + 1)) - 1)
                    vpar = vv[:, c0:c0 + ld].rearrange("p (i t) -> p i t", t=8)
                    dpar = d1[:, c0:c0 + ld].rearrange("p (i t) -> p i t", t=8)
                    kids = vv[:, c1:c1 + 2 * ld].rearrange(
                        "p (i two t) -> p i two t", two=2, t=8)
                    nc.vector.tensor_tensor(kids[:, :, 1, :], vpar, dpar,
                                            Alu.mult)
                    nc.vector.tensor_tensor(kids[:, :, 0, :], vpar,
                                            kids[:, :, 1, :], Alu.subtract)

            def finish_mask(j, ac, vv):
                mk = mks_pool.tile([TT, NODES_PAD], f16, tag="mk")
                nc.vector.memset(mk[:, WIDTH:NODES_PAD], 0.0)
                nc.vector.tensor_tensor(mk[:, 0:1024], ac[:, 0:1024],
                                        vv[:, 0:1024], Alu.mult)
                nc.vector.tensor_tensor(mk[:, 1024:WIDTH], ac[:, 1024:WIDTH],
                                        vv[:, 1024:WIDTH], Alu.mult)
                state[j] = mk

            def gemm1_slab_mm(s, xh, xl, xh2):
                c0, c1 = SLAB_LIM[s]
                w = c1 - c0
                p = pl_pool.tile([TT, 512], f32, tag="pl")
                if s == 0:
                    for k in range(K_CH):
                        nc.tensor.matmul(p, lhsT=xh[:, k, :],
                                         rhs=w1_sb[:, 0, k, :],
                                         start=(k == 0), stop=False)
                    for k in range(K_CH):
                        nc.tensor.matmul(p[:, 0:SH_COLS], lhsT=xl[:, k, :],
                                         rhs=w1_sb[:, 0, k, 0:SH_COLS],
                                         start=False, stop=False)
                    for k in range(K_CH):
                        nc.tensor.matmul(p[:, 0:SH_COLS], lhsT=xh2[:, k, :],
                                         rhs=w1l_sb[:, k, :],
                                         start=False, stop=(k == K_CH - 1))
                else:
                    for k in range(K_CH):
                        nc.tensor.matmul(p[:, 0:w], lhsT=xh[:, k, :],
                                         rhs=w1_sb[:, s, k, 0:w],
                                         start=(k == 0),
                                         stop=(k == K_CH - 1))
                return p

            def gemm1_slab_post(s, p, lg, d1, ac):
                c0, c1 = SLAB_LIM[s]
                w = c1 - c0
                nc.vector.tensor_tensor(lg[:, c0:c1], p[:, 0:w],
                                        b1_sb[:, c0:c1], Alu.add)
                if s == 0:
                    nc.vector.tensor_scalar(d1[:, 0:512], lg[:, 0:512], 0.0,
                                            None, Alu.is_gt)
                elif s == 1:
                    nc.vector.tensor_scalar(d1[:, 512:DEC_COLS],
                                            lg[:, 512:DEC_COLS], 0.0,
                                            None, Alu.is_gt)
                nc.scalar.activation(ac[:, c0:c1], lg[:, c0:c1], Act.Silu)

            def gemm1_slab(s, xh, xl, xh2, lg, d1, ac):
                p = gemm1_slab_mm(s, xh, xl, xh2)
                gemm1_slab_post(s, p, lg, d1, ac)

            def tile_bufs(j):
                xhl = xt_tiles.pop(j)
                xh, xl = xhl[:, 0], xhl[:, 1]
                xh2 = xh2s.tile([128, K_CH, TT], f16, tag="xh2")
                nc.vector.tensor_scalar(xh2, xh, 1.0 / LO_SCALE, None,
                                        Alu.mult)
                lg = logits_pool.tile([TT, WIDTH], f32, tag="lg")
                d1 = mask_pool.tile([TT, DEC_COLS], f16, tag="d1")
                vv = mask_pool.tile([TT, WIDTH], f16, tag="vv")
                ac = acts_pool.tile([TT, WIDTH], f16, tag="ac")
                return xh, xl, xh2, lg, d1, vv, ac

            def stage_a(j):
                if j + 1 < NTILES and j + 1 not in xt_tiles:
                    prefetch_xt(j + 1)
                xh, xl, xh2, lg, d1, vv, ac = tile_bufs(j)
                for s in range(N_SLAB):
                    gemm1_slab(s, xh, xl, xh2, lg, d1, ac)
                epilogue_vec(j, lg, d1, vv, ac)
                finish_mask(j, ac, vv)

            def stage_ab01():
                # tiles 0 and 1 slab-major: each arriving w1 slab feeds
                # 2 tiles of PE work, halving the DMA-bound startup.
                b0 = tile_bufs(0)
                b1_ = tile_bufs(1)
                for s in range(N_SLAB):
                    p0 = gemm1_slab_mm(s, b0[0], b0[1], b0[2])
                    bias_region(s)
                    gemm1_slab_post(s, p0, b0[3], b0[4], b0[6])
                    gemm1_slab(s, b1_[0], b1_[1], b1_[2], b1_[3], b1_[4],
                               b1_[6])
                    if s == 1:
                        epilogue_vec(0, b0[3], b0[4], b0[5], b0[6])
                        epilogue_vec(1, b1_[3], b1_[4], b1_[5], b1_[6])
                finish_mask(0, b0[6], b0[5])
                finish_mask(1, b1_[6], b1_[5])

            def stage_b(j, last=False):
                mk = state.pop(j)
                at = acts_pool.tile([128, C_CH, TT], f16, tag="at")
                c = 0
                for gsz in (1, 2, 3, 4, 3, 3):
                    pt = pt_pool.tile([128, 512], f16)
                    for i in range(gsz):
                        nc.tensor.transpose(
                            pt[:, i * 128:(i + 1) * 128],
                            mk[:, (c + i) * 128:(c + i + 1) * 128], ident)
                    nc.scalar.copy(
                        at[:, c:c + gsz, :],
                        pt[:, :gsz * 128].rearrange("p (c t) -> p c t", t=TT))
                    c += gsz
                ys = out_pool.tile([TT, DIM], f32, tag="ys")
                py0 = py_pool.tile([TT, 512], f32, tag="py")
                py1 = py_pool.tile([TT, 512], f32, tag="py")
                if last:
                    # serialize the halves so the first store drains while
                    # the second half is still on the PE (shorter tail)
                    for h, py in ((0, py0), (1, py1)):
                        hs = slice(h * 512, (h + 1) * 512)
                        for c in range(C_CH):
                            nc.tensor.matmul(py, lhsT=at[:, c, :],
                                             rhs=w2_sb[:, c, hs],
                                             start=(c == 0),
                                             stop=(c == C_CH - 1))
                        nc.vector.tensor_copy(ys[:, hs], py)
                        nc.sync.dma_start(out=y[j * TT:(j + 1) * TT, hs],
                                          in_=ys[:, hs])
                    return
                # c-outer so w2 chunks are consumed in arrival order
                for c in range(C_CH):
                    nc.tensor.matmul(py0, lhsT=at[:, c, :],
                                     rhs=w2_sb[:, c, 0:512],
                                     start=(c == 0), stop=(c == C_CH - 1))
                    nc.tensor.matmul(py1, lhsT=at[:, c, :],
                                     rhs=w2_sb[:, c, 512:1024],
                                     start=(c == 0), stop=(c == C_CH - 1))
                for h, py in ((0, py0), (1, py1)):
                    hs = slice(h * 512, (h + 1) * 512)
                    nc.vector.tensor_copy(ys[:, hs], py)
                    nc.sync.dma_start(out=y[j * TT:(j + 1) * TT, hs],
                                      in_=ys[:, hs])

            # pipeline: AB(0,1), A(2), B(0), A(3), B(1), ... A(7), B(5),
            # B(6), B(7)
            stage_ab01()
            for j in range(2, NTILES):
                stage_a(j)
                stage_b(j - 2)
            stage_b(NTILES - 2)
            stage_b(NTILES - 1, last=True)

    nc.finalize()
    return nc


def _get_program():
    global _PROGRAM
    if _PROGRAM is None:
        _PROGRAM = _build_program()
    return _PROGRAM


def _split_hi_lo_f16(a):
    hi = a.astype(np.float16)
    lo = (a - hi.astype(np.float32)).astype(np.float16)
    return hi, lo


def kernel(oldx, W_in, b_in, W_out):
    from concourse.bass_utils import run_bass_kernel_spmd

    oldx = np.asarray(oldx)
    W_in = np.asarray(W_in, dtype=np.float32)
    b_in = np.asarray(b_in, dtype=np.float32)
    W_out = np.asarray(W_out, dtype=np.float32)
    x = oldx.reshape(-1, DIM).astype(np.float32)          # [8192, 1024]

    # node-major column permutation: our col 8n+t  <-  ref col 255t+n
    i = np.arange(WIDTH)
    perm = 255 * (i % PAR) + (i // PAR)

    w1t = W_in[perm, :].T.astype(np.float32)              # [1024, 2040]
    w1t_hi = w1t.astype(np.float16)
    w1t_lo = ((w1t - w1t_hi.astype(np.float32)) * LO_SCALE).astype(np.float16)
    # [dim, width] -> [128, N_SLAB, K_CH, 512] with dim = k*128 + p,
    # width col = 512*slab + c (last slab zero-padded to 512)
    w1p = np.zeros((1024, N_SLAB * 512), np.float16)
    w1p[:, :WIDTH] = w1t_hi
    w1 = np.ascontiguousarray(
        w1p.reshape(K_CH, 128, N_SLAB, 512).transpose(1, 2, 0, 3))
    w1l = np.ascontiguousarray(
        w1t_lo.reshape(K_CH, 128, WIDTH).transpose(1, 0, 2)[:, :, :SH_COLS])
    b1p = b_in[perm].astype(np.float32)
    b1h = b1p.astype(np.float16)
    b1hl = np.ascontiguousarray(np.stack(
        [b1h, ((b1p - b1h.astype(np.float32)) * LO_SCALE).astype(np.float16)]))

    w2t = np.zeros((NODES_PAD, DIM), np.float32)
    w2t[:WIDTH] = W_out.T[perm, :]
    w2 = np.ascontiguousarray(
        w2t.astype(np.float16).reshape(C_CH, 128, DIM).transpose(1, 0, 2))
    ones2 = np.ascontiguousarray(np.stack(
        [np.full(128, 1.0, np.float16),
         np.full(128, 1.0 / LO_SCALE, np.float16)]))

    in_maps = []
    for c in range(N_CORES):
        xc = x[c * TOK_PER_CORE:(c + 1) * TOK_PER_CORE]   # [1024, 1024]
        xt_hi, xt_lo = _split_hi_lo_f16(xc.T)             # [dim, tok]
        # [dim, tok] -> [128, NTILES, K_CH, TT]; dim = k*128+p, tok = j*128+t
        xt_hi = xt_hi.reshape(K_CH, 128, NTILES, TT).transpose(1, 2, 0, 3)
        xt_lo = xt_lo.reshape(K_CH, 128, NTILES, TT).transpose(1, 2, 0, 3)
        xt = np.ascontiguousarray(np.stack([xt_hi, xt_lo], axis=2))
        in_maps.append({
            "xt": xt, "w1": w1, "w1l": w1l,
            "b1hl": b1hl, "w2": w2, "ones2": ones2,
        })

    nc = _get_program()
    res = run_bass_kernel_spmd(nc, in_maps, core_ids=list(range(N_CORES)))
    out = np.concatenate([res.results[c]["y"] for c in range(N_CORES)],
                         axis=0)
    return out.reshape(oldx.shape).astype(np.float32)


# revision 30
# speedup vs baseline: 1.1843x; 1.0024x over previous
"""Trainium2 Bass kernel for the FFF (fast feedforward / MoE-routing) module.

Math (per token x of dim 1024, PAR=8 trees of 255 nodes):
  logits = x @ W_in.T + b_in                      # [B, 2040]
  dec    = logits > 0
  acts   = silu(logits)
  dmap   = indicator of the 8 visited nodes per tree (root + 7 descents,
           descending by dec at the current node)
  out    = (acts * dmap) @ W_out.T                # [B, 1024]

Strategy (8 NeuronCores, data-parallel over the 8192 tokens, 1024 each):
  - GEMM1 in fp16 with region-dependent precision.  A decision flip at
    depth d corrupts 7-d downstream activations, so only the shallow
    nodes (0..15, levels 0..3) get the expensive treatment:
      cols   0..128 (nodes  0..15): x_hi*w + x_lo*w + x_hi*w_lo  (~fp32)
      cols 128..2040 (everything else): single x_hi*w pass
    The w_lo correction weights are pre-scaled by 2^10 (and x_hi by
    2^-10 on-device) so they stay in fp16 normal range.  fp32 bias is
    added on the vector engine.  Measured on the reference data this
    gives ~1.35e-2 overall rel err (gate is 2e-2).
  - dmap is built level-by-level with strided vector ops in a node-major
    column layout (col = 8*node + tree): child1 = V_d * dec_d (stride-2
    upsample), child0 = V_d - child1.
  - masked acts cast to fp16, transposed 128x128 on the PE, GEMM2 in fp16
    (exact products, fp32 PSUM accumulation).
  - startup: the 4.7MB of GEMM1 weights stream in as four 512-column
    slabs (one dma_start each; the DGE fair-shares ~300GB/s across
    in-flight dispatches, so fewer+ordered dispatches = earliest slab0).
    Tiles 0 and 1 are processed SLAB-MAJOR so each arriving slab feeds
    2 tiles of PE work; the fp32 bias arrives as fp16 (hi, 2^10*lo) rows
    and is broadcast across partitions on the PE (K=2 matmul against a
    (1, 2^-10) column pair -> exact fp32 in PSUM) during the initial
    weight wait instead of a 1MB broadcast DMA competing for early
    bandwidth.
"""

import numpy as np

DIM = 1024
PAR = 8
DEPTH = 7
N_NODES = 255
WIDTH = PAR * N_NODES          # 2040
NODES_PAD = 2048               # pad masked-acts/W_out^T to 16*128
N_CORES = 8
TOK_PER_CORE = 1024
TT = 128                       # tokens per tile
NTILES = TOK_PER_CORE // TT    # 8
K_CH = DIM // 128              # 8 contraction chunks for GEMM1
C_CH = NODES_PAD // 128        # 16 contraction chunks for GEMM2
DEC_COLS = 8 * 127             # 1016: decision nodes are levels 0..6
SH_COLS = 128                  # hi/lo-corrected region: nodes 0..15
LO_SCALE = 1024.0              # 2^10 keeps w_lo out of fp16 subnormals
N_SLAB = 4                     # w1 column slabs of 512 (last is 504+pad)

_PROGRAM = None


def _build_program():
    import concourse.bacc as bacc
    import concourse.tile as tile
    from concourse import mybir
    from concourse.masks import make_identity
    import concourse.bass as bass

    f32 = mybir.dt.float32
    f16 = mybir.dt.float16
    Alu = mybir.AluOpType
    Act = mybir.ActivationFunctionType

    nc = bacc.Bacc("TRN2", target_bir_lowering=False, debug=False,
                   num_devices=N_CORES)

    xt = nc.dram_tensor("xt", [128, NTILES, 2, K_CH, TT], f16,
                        kind="ExternalInput")
    w1 = nc.dram_tensor("w1", [128, N_SLAB, K_CH, 512], f16,
                        kind="ExternalInput")
    w1l = nc.dram_tensor("w1l", [128, K_CH, SH_COLS], f16,
                         kind="ExternalInput")
    b1hl = nc.dram_tensor("b1hl", [2, WIDTH], f16, kind="ExternalInput")
    ones2 = nc.dram_tensor("ones2", [2, 128], f16, kind="ExternalInput")
    w2 = nc.dram_tensor("w2", [128, C_CH, DIM], f16, kind="ExternalInput")
    y = nc.dram_tensor("y", [TOK_PER_CORE, DIM], f32, kind="ExternalOutput")

    SLAB_LIM = [(0, 512), (512, 1024), (1024, 1536), (1536, WIDTH)]

    with tile.TileContext(nc) as tc:
        with (
            tc.tile_pool(name="wts", bufs=1) as wts,
            tc.tile_pool(name="xts", bufs=4) as xts,
            tc.tile_pool(name="xh2s", bufs=2) as xh2s,
            tc.tile_pool(name="logits", bufs=2) as logits_pool,
            tc.tile_pool(name="mask", bufs=2) as mask_pool,
            tc.tile_pool(name="acts", bufs=2) as acts_pool,
            tc.tile_pool(name="mks", bufs=3) as mks_pool,
            tc.tile_pool(name="out", bufs=2) as out_pool,
            tc.tile_pool(name="pl", bufs=3, space="PSUM") as pl_pool,
            tc.tile_pool(name="pt", bufs=3, space="PSUM") as pt_pool,
            tc.tile_pool(name="py", bufs=2, space="PSUM") as py_pool,
        ):
            # ---- resident weights ----
            w1_sb = wts.tile([128, N_SLAB, K_CH, 512], f16)
            w1l_sb = wts.tile([128, K_CH, SH_COLS], f16)
            w2_sb = wts.tile([128, C_CH, DIM], f16)
            b1_sb = wts.tile([128, WIDTH], f32)
            b1_row = wts.tile([2, WIDTH], f16)
            ones = wts.tile([2, 128], f16)
            ident = wts.tile([128, 128], f16)

            xt_tiles = {}

            def prefetch_xt(j):
                xhl = xts.tile([128, 2, K_CH, TT], f16, tag="x")
                nc.sync.dma_start(out=xhl, in_=xt[:, j, :, :, :])
                xt_tiles[j] = xhl

            # Startup DMAs on the Sync engine in PE consumption order.
            # The DGE fair-shares bandwidth over in-flight dispatches and
            # completes them in dispatch order, so this order == arrival
            # order.
            nc.gpsimd.dma_start(out=ones, in_=ones2[:, :])
            nc.gpsimd.dma_start(out=b1_row, in_=b1hl[:, :])
            xhl0 = xts.tile([128, 2, K_CH, TT], f16, tag="x")
            xhl1 = xts.tile([128, 2, K_CH, TT], f16, tag="x")
            nc.scalar.dma_start(out=xhl0[:, 0], in_=xt[:, 0, 0, :, :])
            nc.sync.dma_start(out=w1_sb[:, 0, 0:4], in_=w1[:, 0, 0:4])
            nc.sync.dma_start(out=xhl0[:, 1], in_=xt[:, 0, 1, :, :])
            nc.sync.dma_start(out=w1_sb[:, 0, 4:8], in_=w1[:, 0, 4:8])
            nc.sync.dma_start(out=w1l_sb, in_=w1l[:, :, :])
            nc.sync.dma_start(out=xhl1[:, 0], in_=xt[:, 1, 0, :, :])
            nc.sync.dma_start(out=xhl1[:, 1], in_=xt[:, 1, 1, :, :])
            xt_tiles[0] = xhl0
            xt_tiles[1] = xhl1
            for s in range(1, N_SLAB):
                nc.sync.dma_start(out=w1_sb[:, s], in_=w1[:, s])
            prefetch_xt(2)
            nc.sync.dma_start(out=w2_sb[:, 0:4, :], in_=w2[:, 0:4, :])
            prefetch_xt(3)
            nc.sync.dma_start(out=w2_sb[:, 4:8, :], in_=w2[:, 4:8, :])
            nc.sync.dma_start(out=w2_sb[:, 8:12, :], in_=w2[:, 8:12, :])
            nc.sync.dma_start(out=w2_sb[:, 12:16, :], in_=w2[:, 12:16, :])
            make_identity(nc, ident)

            # bias broadcast across partitions on the PE: fp16 (hi,
            # 2^10*lo) rows contracted (K=2) against (1, 2^-10) -> exact
            # fp32 in PSUM.  Emitted per-region inside stage_ab01, right
            # after each slab's first tile, so the PE's first instruction
            # is GEMM1 itself (gated only on x + slab0) and each bias mm
            # slots into a DMA gap.
            def bias_region(s):
                c0, c1 = SLAB_LIM[s]
                pb = pl_pool.tile([TT, 512], f32, tag="pl")
                nc.tensor.matmul(pb[:, 0:c1 - c0], lhsT=ones,
                                 rhs=b1_row[:, c0:c1], start=True, stop=True)
                nc.vector.tensor_copy(b1_sb[:, c0:c1], pb[:, 0:c1 - c0])

            state = {}

            def epilogue_vec(j, lg, d1, vv, ac):
                # tree mask: V_0 = 1 at root cols; then per level
                # child1 = V_d * dec_d, child0 = V_d - child1
                nc.vector.memset(vv[:, 0:8], 1.0)
                for d in range(DEPTH):
                    ld = 8 * (1 << d)
                    c0 = 8 * ((1 << d) - 1)
                    c1 = 8 * ((1 << (d + 1)) - 1)
                    vpar = vv[:, c0:c0 + ld].rearrange("p (i t) -> p i t", t=8)
                    dpar = d1[:, c0:c0 + ld].rearrange("p (i t) -> p i t", t=8)
                    kids = vv[:, c1:c1 + 2 * ld].rearrange(
                        "p (i two t) -> p i two t", two=2, t=8)
                    nc.vector.tensor_tensor(kids[:, :, 1, :], vpar, dpar,
                                            Alu.mult)
                    nc.vector.tensor_tensor(kids[:, :, 0, :], vpar,
                                            kids[:, :, 1, :], Alu.subtract)

            def finish_mask(j, ac, vv):
                mk = mks_pool.tile([TT, NODES_PAD], f16, tag="mk")
                nc.vector.memset(mk[:, WIDTH:NODES_PAD], 0.0)
                nc.vector.tensor_tensor(mk[:, 0:1024], ac[:, 0:1024],
                                        vv[:, 0:1024], Alu.mult)
                nc.vector.tensor_tensor(mk[:, 1024:WIDTH], ac[:, 1024:WIDTH],
                                        vv[:, 1024:WIDTH], Alu.mult)
                state[j] = mk

            def gemm1_slab_mm(s, xh, xl, xh2):
                c0, c1 = SLAB_LIM[s]
                w = c1 - c0
                p = pl_pool.tile([TT, 512], f32, tag="pl")
                if s == 0:
                    for k in range(K_CH):
                        nc.tensor.matmul(p, lhsT=xh[:, k, :],
                                         rhs=w1_sb[:, 0, k, :],
                                         start=(k == 0), stop=False)
                    for k in range(K_CH):
                        nc.tensor.matmul(p[:, 0:SH_COLS], lhsT=xl[:, k, :],
                                         rhs=w1_sb[:, 0, k, 0:SH_COLS],
                                         start=False, stop=False)
                    for k in range(K_CH):
                        nc.tensor.matmul(p[:, 0:SH_COLS], lhsT=xh2[:, k, :],
                                         rhs=w1l_sb[:, k, :],
                                         start=False, stop=(k == K_CH - 1))
                else:
                    for k in range(K_CH):
                        nc.tensor.matmul(p[:, 0:w], lhsT=xh[:, k, :],
                                         rhs=w1_sb[:, s, k, 0:w],
                                         start=(k == 0),
                                         stop=(k == K_CH - 1))
                return p

            def gemm1_slab_post(s, p, lg, d1, ac):
                c0, c1 = SLAB_LIM[s]
                w = c1 - c0
                nc.vector.tensor_tensor(lg[:, c0:c1], p[:, 0:w],
                                        b1_sb[:, c0:c1], Alu.add)
                if s == 0:
                    nc.vector.tensor_scalar(d1[:, 0:512], lg[:, 0:512], 0.0,
                                            None, Alu.is_gt)
                elif s == 1:
                    nc.vector.tensor_scalar(d1[:, 512:DEC_COLS],
                                            lg[:, 512:DEC_COLS], 0.0,
                                            None, Alu.is_gt)
                nc.scalar.activation(ac[:, c0:c1], lg[:, c0:c1], Act.Silu)

            def gemm1_slab(s, xh, xl, xh2, lg, d1, ac):
                p = gemm1_slab_mm(s, xh, xl, xh2)
                gemm1_slab_post(s, p, lg, d1, ac)

            def tile_bufs(j):
                xhl = xt_tiles.pop(j)
                xh, xl = xhl[:, 0], xhl[:, 1]
                xh2 = xh2s.tile([128, K_CH, TT], f16, tag="xh2")
                nc.vector.tensor_scalar(xh2, xh, 1.0 / LO_SCALE, None,
                                        Alu.mult)
                lg = logits_pool.tile([TT, WIDTH], f32, tag="lg")
                d1 = mask_pool.tile([TT, DEC_COLS], f16, tag="d1")
                vv = mask_pool.tile([TT, WIDTH], f16, tag="vv")
                ac = acts_pool.tile([TT, WIDTH], f16, tag="ac")
                return xh, xl, xh2, lg, d1, vv, ac

            def stage_a(j):
                if j + 1 < NTILES and j + 1 not in xt_tiles:
                    prefetch_xt(j + 1)
                xh, xl, xh2, lg, d1, vv, ac = tile_bufs(j)
                for s in range(N_SLAB):
                    gemm1_slab(s, xh, xl, xh2, lg, d1, ac)
                epilogue_vec(j, lg, d1, vv, ac)
                finish_mask(j, ac, vv)

            def stage_ab01():
                # tiles 0 and 1 slab-major: each arriving w1 slab feeds
                # 2 tiles of PE work, halving the DMA-bound startup.
                b0 = tile_bufs(0)
                b1_ = tile_bufs(1)
                for s in range(N_SLAB):
                    p0 = gemm1_slab_mm(s, b0[0], b0[1], b0[2])
                    bias_region(s)
                    gemm1_slab_post(s, p0, b0[3], b0[4], b0[6])
                    gemm1_slab(s, b1_[0], b1_[1], b1_[2], b1_[3], b1_[4],
                               b1_[6])
                    if s == 1:
                        epilogue_vec(0, b0[3], b0[4], b0[5], b0[6])
                        epilogue_vec(1, b1_[3], b1_[4], b1_[5], b1_[6])
                finish_mask(0, b0[6], b0[5])
                finish_mask(1, b1_[6], b1_[5])

            def stage_b(j, last=False):
                mk = state.pop(j)
                at = acts_pool.tile([128, C_CH, TT], f16, tag="at")
                c = 0
                for gsz in (1, 2, 3, 4, 3, 3):
                    pt = pt_pool.tile([128, 512], f16)
                    for i in range(gsz):
                        nc.tensor.transpose(
                            pt[:, i * 128:(i + 1) * 128],
                            mk[:, (c + i) * 128:(c + i + 1) * 128], ident)
                    nc.scalar.copy(
                        at[:, c:c + gsz, :],
                        pt[:, :gsz * 128].rearrange("p (c t) -> p c t", t=TT))
                    c += gsz
                ys = out_pool.tile([TT, DIM], f32, tag="ys")
                py0 = py_pool.tile([TT, 512], f32, tag="py")
                py1 = py_pool.tile([TT, 512], f32, tag="py")
                if last:
                    # serialize the halves so the first store drains while
                    # the second half is still on the PE (shorter tail)
                    for h, py in ((0, py0), (1, py1)):
                        hs = slice(h * 512, (h + 1) * 512)
                        for c in range(C_CH):
                            nc.tensor.matmul(py, lhsT=at[:, c, :],
                                             rhs=w2_sb[:, c, hs],
                                             start=(c == 0),
                                             stop=(c == C_CH - 1))
                        nc.vector.tensor_copy(ys[:, hs], py)
                        nc.sync.dma_start(out=y[j * TT:(j + 1) * TT, hs],
                                          in_=ys[:, hs])
                    return
                # c-outer so w2 chunks are consumed in arrival order
                for c in range(C_CH):
                    nc.tensor.matmul(py0, lhsT=at[:, c, :],
                                     rhs=w2_sb[:, c, 0:512],
                                     start=(c == 0), stop=(c == C_CH - 1))
                    nc.tensor.matmul(py1, lhsT=at[:, c, :],
                                     rhs=w2_sb[:, c, 512:1024],
                                     start=(c == 0), stop=(c == C_CH - 1))
                for h, py in ((0, py0), (1, py1)):
                    hs = slice(h * 512, (h + 1) * 512)
                    nc.vector.tensor_copy(ys[:, hs], py)
                    nc.sync.dma_start(out=y[j * TT:(j + 1) * TT, hs],
                                      in_=ys[:, hs])

            # pipeline: AB(0,1), A(2), B(0), A(3), B(1), ... A(7), B(5),
            # B(6), B(7)
            stage_ab01()
            for j in range(2, NTILES):
                stage_a(j)
                stage_b(j - 2)
            stage_b(NTILES - 2)
            stage_b(NTILES - 1, last=True)

    nc.finalize()
    return nc


def _get_program():
    global _PROGRAM
    if _PROGRAM is None:
        _PROGRAM = _build_program()
    return _PROGRAM


def _split_hi_lo_f16(a):
    hi = a.astype(np.float16)
    lo = (a - hi.astype(np.float32)).astype(np.float16)
    return hi, lo


def kernel(oldx, W_in, b_in, W_out):
    from concourse.bass_utils import run_bass_kernel_spmd

    oldx = np.asarray(oldx)
    W_in = np.asarray(W_in, dtype=np.float32)
    b_in = np.asarray(b_in, dtype=np.float32)
    W_out = np.asarray(W_out, dtype=np.float32)
    x = oldx.reshape(-1, DIM).astype(np.float32)          # [8192, 1024]

    # node-major column permutation: our col 8n+t  <-  ref col 255t+n
    i = np.arange(WIDTH)
    perm = 255 * (i % PAR) + (i // PAR)

    w1t = W_in[perm, :].T.astype(np.float32)              # [1024, 2040]
    w1t_hi = w1t.astype(np.float16)
    w1t_lo = ((w1t - w1t_hi.astype(np.float32)) * LO_SCALE).astype(np.float16)
    # [dim, width] -> [128, N_SLAB, K_CH, 512] with dim = k*128 + p,
    # width col = 512*slab + c (last slab zero-padded to 512)
    w1p = np.zeros((1024, N_SLAB * 512), np.float16)
    w1p[:, :WIDTH] = w1t_hi
    w1 = np.ascontiguousarray(
        w1p.reshape(K_CH, 128, N_SLAB, 512).transpose(1, 2, 0, 3))
    w1l = np.ascontiguousarray(
        w1t_lo.reshape(K_CH, 128, WIDTH).transpose(1, 0, 2)[:, :, :SH_COLS])
    b1p = b_in[perm].astype(np.float32)
    b1h = b1p.astype(np.float16)
    b1hl = np.ascontiguousarray(np.stack(
        [b1h, ((b1p - b1h.astype(np.float32)) * LO_SCALE).astype(np.float16)]))

    w2t = np.zeros((NODES_PAD, DIM), np.float32)
    w2t[:WIDTH] = W_out.T[perm, :]
    w2 = np.ascontiguousarray(
        w2t.astype(np.float16).reshape(C_CH, 128, DIM).transpose(1, 0, 2))
    ones2 = np.ascontiguousarray(np.stack(
        [np.full(128, 1.0, np.float16),
         np.full(128, 1.0 / LO_SCALE, np.float16)]))

    in_maps = []
    for c in range(N_CORES):
        xc = x[c * TOK_PER_CORE:(c + 1) * TOK_PER_CORE]   # [1024, 1024]
        xt_hi, xt_lo = _split_hi_lo_f16(xc.T)             # [dim, tok]
        # [dim, tok] -> [128, NTILES, K_CH, TT]; dim = k*128+p, tok = j*128+t
        xt_hi = xt_hi.reshape(K_CH, 128, NTILES, TT).transpose(1, 2, 0, 3)
        xt_lo = xt_lo.reshape(K_CH, 128, NTILES, TT).transpose(1, 2, 0, 3)
        xt = np.ascontiguousarray(np.stack([xt_hi, xt_lo], axis=2))
        in_maps.append({
            "xt": xt, "w1": w1, "w1l": w1l,
            "b1hl": b1hl, "w2": w2, "ones2": ones2,
        })

    nc = _get_program()
    res = run_bass_kernel_spmd(nc, in_maps, core_ids=list(range(N_CORES)))
    out = np.concatenate([res.results[c]["y"] for c in range(N_CORES)],
                         axis=0)
    return out.reshape(oldx.shape).astype(np.float32)


# revision 31
# speedup vs baseline: 1.1877x; 1.0029x over previous
"""Trainium2 Bass kernel for the FFF (fast feedforward / MoE-routing) module.

Math (per token x of dim 1024, PAR=8 trees of 255 nodes):
  logits = x @ W_in.T + b_in                      # [B, 2040]
  dec    = logits > 0
  acts   = silu(logits)
  dmap   = indicator of the 8 visited nodes per tree (root + 7 descents,
           descending by dec at the current node)
  out    = (acts * dmap) @ W_out.T                # [B, 1024]

Strategy (8 NeuronCores, data-parallel over the 8192 tokens, 1024 each):
  - GEMM1 in fp16 with region-dependent precision.  A decision flip at
    depth d corrupts 7-d downstream activations, so only the shallow
    nodes (0..15, levels 0..3) get the expensive treatment:
      cols   0..128 (nodes  0..15): x_hi*w + x_lo*w + x_hi*w_lo  (~fp32)
      cols 128..2040 (everything else): single x_hi*w pass
    The w_lo correction weights are pre-scaled by 2^10 (and x_hi by
    2^-10 on-device) so they stay in fp16 normal range.  fp32 bias is
    added on the vector engine.  Measured on the reference data this
    gives ~1.35e-2 overall rel err (gate is 2e-2).
  - dmap is built level-by-level with strided vector ops in a node-major
    column layout (col = 8*node + tree): child1 = V_d * dec_d (stride-2
    upsample), child0 = V_d - child1.
  - masked acts cast to fp16, transposed 128x128 on the PE, GEMM2 in fp16
    (exact products, fp32 PSUM accumulation).
  - startup: the 4.7MB of GEMM1 weights stream in as four 512-column
    slabs (one dma_start each; the DGE fair-shares ~300GB/s across
    in-flight dispatches, so fewer+ordered dispatches = earliest slab0).
    Tiles 0 and 1 are processed SLAB-MAJOR so each arriving slab feeds
    2 tiles of PE work; the fp32 bias arrives as fp16 (hi, 2^10*lo) rows
    and is broadcast across partitions on the PE (K=2 matmul against a
    (1, 2^-10) column pair -> exact fp32 in PSUM) during the initial
    weight wait instead of a 1MB broadcast DMA competing for early
    bandwidth.
"""

import numpy as np

DIM = 1024
PAR = 8
DEPTH = 7
N_NODES = 255
WIDTH = PAR * N_NODES          # 2040
NODES_PAD = 2048               # pad masked-acts/W_out^T to 16*128
N_CORES = 8
TOK_PER_CORE = 1024
TT = 128                       # tokens per tile
NTILES = TOK_PER_CORE // TT    # 8
K_CH = DIM // 128              # 8 contraction chunks for GEMM1
C_CH = NODES_PAD // 128        # 16 contraction chunks for GEMM2
DEC_COLS = 8 * 127             # 1016: decision nodes are levels 0..6
SH_COLS = 128                  # hi/lo-corrected region: nodes 0..15
LO_SCALE = 1024.0              # 2^10 keeps w_lo out of fp16 subnormals
N_SLAB = 4                     # w1 column slabs of 512 (last is 504+pad)

_PROGRAM = None


def _build_program():
    import concourse.bacc as bacc
    import concourse.tile as tile
    from concourse import mybir
    from concourse.masks import make_identity
    import concourse.bass as bass

    f32 = mybir.dt.float32
    f16 = mybir.dt.float16
    Alu = mybir.AluOpType
    Act = mybir.ActivationFunctionType

    nc = bacc.Bacc("TRN2", target_bir_lowering=False, debug=False,
                   num_devices=N_CORES)

    xt = nc.dram_tensor("xt", [128, NTILES, 2, K_CH, TT], f16,
                        kind="ExternalInput")
    w1 = nc.dram_tensor("w1", [128, N_SLAB, K_CH, 512], f16,
                        kind="ExternalInput")
    w1l = nc.dram_tensor("w1l", [128, K_CH, SH_COLS], f16,
                         kind="ExternalInput")
    b1hl = nc.dram_tensor("b1hl", [2, WIDTH], f16, kind="ExternalInput")
    ones2 = nc.dram_tensor("ones2", [2, 128], f16, kind="ExternalInput")
    w2 = nc.dram_tensor("w2", [128, C_CH, DIM], f16, kind="ExternalInput")
    y = nc.dram_tensor("y", [TOK_PER_CORE, DIM], f32, kind="ExternalOutput")

    SLAB_LIM = [(0, 512), (512, 1024), (1024, 1536), (1536, WIDTH)]

    with tile.TileContext(nc) as tc:
        with (
            tc.tile_pool(name="wts", bufs=1) as wts,
            tc.tile_pool(name="xts", bufs=4) as xts,
            tc.tile_pool(name="xh2s", bufs=2) as xh2s,
            tc.tile_pool(name="logits", bufs=2) as logits_pool,
            tc.tile_pool(name="mask", bufs=2) as mask_pool,
            tc.tile_pool(name="acts", bufs=2) as acts_pool,
            tc.tile_pool(name="mks", bufs=3) as mks_pool,
            tc.tile_pool(name="out", bufs=2) as out_pool,
            tc.tile_pool(name="pl", bufs=3, space="PSUM") as pl_pool,
            tc.tile_pool(name="pt", bufs=3, space="PSUM") as pt_pool,
            tc.tile_pool(name="py", bufs=2, space="PSUM") as py_pool,
        ):
            # ---- resident weights ----
            w1_sb = wts.tile([128, N_SLAB, K_CH, 512], f16)
            w1l_sb = wts.tile([128, K_CH, SH_COLS], f16)
            w2_sb = wts.tile([128, C_CH, DIM], f16)
            b1_sb = wts.tile([128, WIDTH], f32)
            b1_row = wts.tile([2, WIDTH], f16)
            ones = wts.tile([2, 128], f16)
            ident = wts.tile([128, 128], f16)

            xt_tiles = {}

            def prefetch_xt(j):
                xhl = xts.tile([128, 2, K_CH, TT], f16, tag="x")
                nc.sync.dma_start(out=xhl, in_=xt[:, j, :, :, :])
                xt_tiles[j] = xhl

            # Startup DMAs on the Sync engine in PE consumption order.
            # The DGE fair-shares bandwidth over in-flight dispatches and
            # completes them in dispatch order, so this order == arrival
            # order.
            nc.gpsimd.dma_start(out=ones, in_=ones2[:, :])
            nc.gpsimd.dma_start(out=b1_row, in_=b1hl[:, :])
            xhl0 = xts.tile([128, 2, K_CH, TT], f16, tag="x")
            xhl1 = xts.tile([128, 2, K_CH, TT], f16, tag="x")
            nc.sync.dma_start(out=xhl0[:, 0], in_=xt[:, 0, 0, :, :])
            nc.sync.dma_start(out=w1_sb[:, 0, 0:4], in_=w1[:, 0, 0:4])
            nc.sync.dma_start(out=xhl0[:, 1], in_=xt[:, 0, 1, :, :])
            nc.sync.dma_start(out=w1_sb[:, 0, 4:8], in_=w1[:, 0, 4:8])
            nc.sync.dma_start(out=w1l_sb, in_=w1l[:, :, :])
            nc.sync.dma_start(out=xhl1[:, 0], in_=xt[:, 1, 0, :, :])
            nc.sync.dma_start(out=xhl1[:, 1], in_=xt[:, 1, 1, :, :])
            xt_tiles[0] = xhl0
            xt_tiles[1] = xhl1
            for s in range(1, N_SLAB):
                nc.sync.dma_start(out=w1_sb[:, s], in_=w1[:, s])
            prefetch_xt(2)
            nc.sync.dma_start(out=w2_sb[:, 0:4, :], in_=w2[:, 0:4, :])
            prefetch_xt(3)
            nc.sync.dma_start(out=w2_sb[:, 4:8, :], in_=w2[:, 4:8, :])
            nc.sync.dma_start(out=w2_sb[:, 8:12, :], in_=w2[:, 8:12, :])
            nc.sync.dma_start(out=w2_sb[:, 12:16, :], in_=w2[:, 12:16, :])
            make_identity(nc, ident)

            # bias broadcast across partitions on the PE: fp16 (hi,
            # 2^10*lo) rows contracted (K=2) against (1, 2^-10) -> exact
            # fp32 in PSUM.  Emitted per-region inside stage_ab01, right
            # after each slab's first tile, so the PE's first instruction
            # is GEMM1 itself (gated only on x + slab0) and each bias mm
            # slots into a DMA gap.
            def bias_region(s):
                c0, c1 = SLAB_LIM[s]
                pb = pl_pool.tile([TT, 512], f32, tag="pl")
                nc.tensor.matmul(pb[:, 0:c1 - c0], lhsT=ones,
                                 rhs=b1_row[:, c0:c1], start=True, stop=True)
                nc.vector.tensor_copy(b1_sb[:, c0:c1], pb[:, 0:c1 - c0])

            state = {}

            def epilogue_vec(j, lg, d1, vv, ac):
                # tree mask: V_0 = 1 at root cols; then per level
                # child1 = V_d * dec_d, child0 = V_d - child1
                nc.vector.memset(vv[:, 0:8], 1.0)
                for d in range(DEPTH):
                    ld = 8 * (1 << d)
                    c0 = 8 * ((1 << d) - 1)
                    c1 = 8 * ((1 << (d + 1)) - 1)
                    vpar = vv[:, c0:c0 + ld].rearrange("p (i t) -> p i t", t=8)
                    dpar = d1[:, c0:c0 + ld].rearrange("p (i t) -> p i t", t=8)
                    kids = vv[:, c1:c1 + 2 * ld].rearrange(
                        "p (i two t) -> p i two t", two=2, t=8)
                    nc.vector.tensor_tensor(kids[:, :, 1, :], vpar, dpar,
                                            Alu.mult)
                    nc.vector.tensor_tensor(kids[:, :, 0, :], vpar,
                                            kids[:, :, 1, :], Alu.subtract)

            def finish_mask(j, ac, vv):
                mk = mks_pool.tile([TT, NODES_PAD], f16, tag="mk")
                nc.vector.memset(mk[:, WIDTH:NODES_PAD], 0.0)
                nc.vector.tensor_tensor(mk[:, 0:1024], ac[:, 0:1024],
                                        vv[:, 0:1024], Alu.mult)
                nc.vector.tensor_tensor(mk[:, 1024:WIDTH], ac[:, 1024:WIDTH],
                                        vv[:, 1024:WIDTH], Alu.mult)
                state[j] = mk

            def gemm1_slab_mm(s, xh, xl, xh2):
                c0, c1 = SLAB_LIM[s]
                w = c1 - c0
                p = pl_pool.tile([TT, 512], f32, tag="pl")
                if s == 0:
                    for k in range(K_CH):
                        nc.tensor.matmul(p, lhsT=xh[:, k, :],
                                         rhs=w1_sb[:, 0, k, :],
                                         start=(k == 0), stop=False)
                    for k in range(K_CH):
                        nc.tensor.matmul(p[:, 0:SH_COLS], lhsT=xl[:, k, :],
                                         rhs=w1_sb[:, 0, k, 0:SH_COLS],
                                         start=False, stop=False)
                    for k in range(K_CH):
                        nc.tensor.matmul(p[:, 0:SH_COLS], lhsT=xh2[:, k, :],
                                         rhs=w1l_sb[:, k, :],
                                         start=False, stop=(k == K_CH - 1))
                else:
                    for k in range(K_CH):
                        nc.tensor.matmul(p[:, 0:w], lhsT=xh[:, k, :],
                                         rhs=w1_sb[:, s, k, 0:w],
                                         start=(k == 0),
                                         stop=(k == K_CH - 1))
                return p

            def gemm1_slab_post(s, p, lg, d1, ac):
                c0, c1 = SLAB_LIM[s]
                w = c1 - c0
                nc.vector.tensor_tensor(lg[:, c0:c1], p[:, 0:w],
                                        b1_sb[:, c0:c1], Alu.add)
                if s == 0:
                    nc.vector.tensor_scalar(d1[:, 0:512], lg[:, 0:512], 0.0,
                                            None, Alu.is_gt)
                elif s == 1:
                    nc.vector.tensor_scalar(d1[:, 512:DEC_COLS],
                                            lg[:, 512:DEC_COLS], 0.0,
                                            None, Alu.is_gt)
                nc.scalar.activation(ac[:, c0:c1], lg[:, c0:c1], Act.Silu)

            def gemm1_slab(s, xh, xl, xh2, lg, d1, ac):
                p = gemm1_slab_mm(s, xh, xl, xh2)
                gemm1_slab_post(s, p, lg, d1, ac)

            def tile_bufs(j):
                xhl = xt_tiles.pop(j)
                xh, xl = xhl[:, 0], xhl[:, 1]
                xh2 = xh2s.tile([128, K_CH, TT], f16, tag="xh2")
                nc.vector.tensor_scalar(xh2, xh, 1.0 / LO_SCALE, None,
                                        Alu.mult)
                lg = logits_pool.tile([TT, WIDTH], f32, tag="lg")
                d1 = mask_pool.tile([TT, DEC_COLS], f16, tag="d1")
                vv = mask_pool.tile([TT, WIDTH], f16, tag="vv")
                ac = acts_pool.tile([TT, WIDTH], f16, tag="ac")
                return xh, xl, xh2, lg, d1, vv, ac

            def stage_a(j):
                if j + 1 < NTILES and j + 1 not in xt_tiles:
                    prefetch_xt(j + 1)
                xh, xl, xh2, lg, d1, vv, ac = tile_bufs(j)
                for s in range(N_SLAB):
                    gemm1_slab(s, xh, xl, xh2, lg, d1, ac)
                epilogue_vec(j, lg, d1, vv, ac)
                finish_mask(j, ac, vv)

            def stage_ab01():
                # tiles 0 and 1 slab-major: each arriving w1 slab feeds
                # 2 tiles of PE work, halving the DMA-bound startup.
                b0 = tile_bufs(0)
                b1_ = tile_bufs(1)
                for s in range(N_SLAB):
                    p0 = gemm1_slab_mm(s, b0[0], b0[1], b0[2])
                    bias_region(s)
                    gemm1_slab_post(s, p0, b0[3], b0[4], b0[6])
                    gemm1_slab(s, b1_[0], b1_[1], b1_[2], b1_[3], b1_[4],
                               b1_[6])
                    if s == 1:
                        epilogue_vec(0, b0[3], b0[4], b0[5], b0[6])
                        epilogue_vec(1, b1_[3], b1_[4], b1_[5], b1_[6])
                finish_mask(0, b0[6], b0[5])
                finish_mask(1, b1_[6], b1_[5])

            def stage_b(j, last=False):
                mk = state.pop(j)
                at = acts_pool.tile([128, C_CH, TT], f16, tag="at")
                c = 0
                for gsz in (1, 2, 3, 4, 3, 3):
                    pt = pt_pool.tile([128, 512], f16)
                    for i in range(gsz):
                        nc.tensor.transpose(
                            pt[:, i * 128:(i + 1) * 128],
                            mk[:, (c + i) * 128:(c + i + 1) * 128], ident)
                    nc.scalar.copy(
                        at[:, c:c + gsz, :],
                        pt[:, :gsz * 128].rearrange("p (c t) -> p c t", t=TT))
                    c += gsz
                ys = out_pool.tile([TT, DIM], f32, tag="ys")
                py0 = py_pool.tile([TT, 512], f32, tag="py")
                py1 = py_pool.tile([TT, 512], f32, tag="py")
                if last:
                    # serialize the halves so the first store drains while
                    # the second half is still on the PE (shorter tail)
                    for h, py in ((0, py0), (1, py1)):
                        hs = slice(h * 512, (h + 1) * 512)
                        for c in range(C_CH):
                            nc.tensor.matmul(py, lhsT=at[:, c, :],
                                             rhs=w2_sb[:, c, hs],
                                             start=(c == 0),
                                             stop=(c == C_CH - 1))
                        nc.vector.tensor_copy(ys[:, hs], py)
                        nc.sync.dma_start(out=y[j * TT:(j + 1) * TT, hs],
                                          in_=ys[:, hs])
                    return
                # c-outer so w2 chunks are consumed in arrival order
                for c in range(C_CH):
                    nc.tensor.matmul(py0, lhsT=at[:, c, :],
                                     rhs=w2_sb[:, c, 0:512],
                                     start=(c == 0), stop=(c == C_CH - 1))
                    nc.tensor.matmul(py1, lhsT=at[:, c, :],
                                     rhs=w2_sb[:, c, 512:1024],
                                     start=(c == 0), stop=(c == C_CH - 1))
                for h, py in ((0, py0), (1, py1)):
                    hs = slice(h * 512, (h + 1) * 512)
                    nc.vector.tensor_copy(ys[:, hs], py)
                    nc.sync.dma_start(out=y[j * TT:(j + 1) * TT, hs],
                                      in_=ys[:, hs])

            # pipeline: AB(0,1), A(2), B(0), A(3), B(1), ... A(7), B(5),
            # B(6), B(7)
            stage_ab01()
            for j in range(2, NTILES):
                stage_a(j)
                stage_b(j - 2)
            stage_b(NTILES - 2)
            stage_b(NTILES - 1, last=True)

    nc.finalize()
    return nc


def _get_program():
    global _PROGRAM
    if _PROGRAM is None:
        _PROGRAM = _build_program()
    return _PROGRAM


def _split_hi_lo_f16(a):
    hi = a.astype(np.float16)
    lo = (a - hi.astype(np.float32)).astype(np.float16)
    return hi, lo


def kernel(oldx, W_in, b_in, W_out):
    from concourse.bass_utils import run_bass_kernel_spmd

    oldx = np.asarray(oldx)
    W_in = np.asarray(W_in, dtype=np.float32)
    b_in = np.asarray(b_in, dtype=np.float32)
    W_out = np.asarray(W_out, dtype=np.float32)
    x = oldx.reshape(-1, DIM).astype(np.float32)          # [8192, 1024]

    # node-major column permutation: our col 8n+t  <-  ref col 255t+n
    i = np.arange(WIDTH)
    perm = 255 * (i % PAR) + (i // PAR)

    w1t = W_in[perm, :].T.astype(np.float32)              # [1024, 2040]
    w1t_hi = w1t.astype(np.float16)
    w1t_lo = ((w1t - w1t_hi.astype(np.float32)) * LO_SCALE).astype(np.float16)
    # [dim, width] -> [128, N_SLAB, K_CH, 512] with dim = k*128 + p,
    # width col = 512*slab + c (last slab zero-padded to 512)
    w1p = np.zeros((1024, N_SLAB * 512), np.float16)
    w1p[:, :WIDTH] = w1t_hi
    w1 = np.ascontiguousarray(
        w1p.reshape(K_CH, 128, N_SLAB, 512).transpose(1, 2, 0, 3))
    w1l = np.ascontiguousarray(
        w1t_lo.reshape(K_CH, 128, WIDTH).transpose(1, 0, 2)[:, :, :SH_COLS])
    b1p = b_in[perm].astype(np.float32)
    b1h = b1p.astype(np.float16)
    b1hl = np.ascontiguousarray(np.stack(
        [b1h, ((b1p - b1h.astype(np.float32)) * LO_SCALE).astype(np.float16)]))

    w2t = np.zeros((NODES_PAD, DIM), np.float32)
    w2t[:WIDTH] = W_out.T[perm, :]
    w2 = np.ascontiguousarray(
        w2t.astype(np.float16).reshape(C_CH, 128, DIM).transpose(1, 0, 2))
    ones2 = np.ascontiguousarray(np.stack(
        [np.full(128, 1.0, np.float16),
         np.full(128, 1.0 / LO_SCALE, np.float16)]))

    in_maps = []
    for c in range(N_CORES):
        xc = x[c * TOK_PER_CORE:(c + 1) * TOK_PER_CORE]   # [1024, 1024]
        xt_hi, xt_lo = _split_hi_lo_f16(xc.T)             # [dim, tok]
        # [dim, tok] -> [128, NTILES, K_CH, TT]; dim = k*128+p, tok = j*128+t
        xt_hi = xt_hi.reshape(K_CH, 128, NTILES, TT).transpose(1, 2, 0, 3)
        xt_lo = xt_lo.reshape(K_CH, 128, NTILES, TT).transpose(1, 2, 0, 3)
        xt = np.ascontiguousarray(np.stack([xt_hi, xt_lo], axis=2))
        in_maps.append({
            "xt": xt, "w1": w1, "w1l": w1l,
            "b1hl": b1hl, "w2": w2, "ones2": ones2,
        })

    nc = _get_program()
    res = run_bass_kernel_spmd(nc, in_maps, core_ids=list(range(N_CORES)))
    out = np.concatenate([res.results[c]["y"] for c in range(N_CORES)],
                         axis=0)
    return out.reshape(oldx.shape).astype(np.float32)
